# revision 51
# baseline (speedup 1.0000x reference)
"""Affinity-propagation (CSPN-3D) Trainium2 kernel, v3.

Problem: guidance [24,256,256,32] f32, blur [1,256,256,32] f32.
3 iterations of (x-plane, y-plane, z-plane) 8-neighbor gated propagation:

  out(q) = r(q) + c1(q) * [ sum_k G_k(q+d_k) * r(q+d_k) - S(q) * r(q) ]
  A(q) = sum_k |G_k(q+d_k)|, S(q) = sum_k G_k(q+d_k), c1 = 1/max(A,eps)

Sharding: 8 cores, X sharded 32 rows/core, ghost margin 5 rows each side;
step 1 consumes no margin (host supplies +-1-x-shifted blur slabs and
unbaked x gates), the remaining 5 x-crossing steps consume 1 each.

Per-core layout: partitions p = yb*42 + xl (3 y-blocks x 42 x-rows = 126);
free f = ylocal*32 + z, ylocal in [0,88) (86-wide y third + 1 overlap col
each side), z in [0,32) unpadded (z boundary handled by zero gates).
FD = 2816, chunked [512 x 5, 256].

Gates are host-pre-shifted by their full 3D offset and, for the x/y axes,
additionally "baked" by -da along partitions so products are computed
against the partition-local rc copy; a PE matmul with a +-1-shift
stationary routes each da group back while accumulating all 9 slots
(8 gates + a -S center slot) into PSUM. Per step:
 - Act: rc = bf16(r) into a guarded window buffer; psum -> bf16 drain.
 - DVE: one 9-slot product instruction per chunk (windowed 4-dim AP).
 - PE: 9 matmuls (3 stationaries) accumulate slots into PSUM f32.
 - Pool: t = c1*psum_bf16 ; r += t.
A/S/c1 are computed on device: |g| via 4x-mode bitmask (DVE), slot sums
via the same PE routing, 1/A via DVE reciprocal.
"""

import numpy as np
import ml_dtypes

BF = ml_dtypes.bfloat16

X = Y = 256
Z = 32
NCORES = 8
W = X // NCORES          # 32 interior x rows per core
M = 5                    # ghost margin rows
S = W + 2 * M            # 42 slab rows
NYB = 3                  # y thirds
YT = 86                  # y third width
YC = YT + 4              # y cols incl 2 overlap each side
ZC = Z                   # z cols, unpadded
FD = YC * ZC             # 2880
P = NYB * S              # 126 partitions
CHUNKS = [(0, 512), (512, 512), (1024, 512), (1536, 512),
          (2048, 512), (2560, 320)]
NCH = len(CHUNKS)
GUARD = 34               # window guard (max offset ZC+1=33)
SLOTF = GUARD + FD + GUARD   # 2884
PROP_TIME = 3
EPS = 1e-30

# k -> (dH, dW) neighbor offsets, matching reference PADS
DLIST = [(1, 1), (1, 0), (1, -1), (0, 1), (0, -1), (-1, 1), (-1, 0), (-1, -1)]
# 3x3 slot enumeration (da, db), row-major; center (0,0) is the nS slot.
SLOT33 = [(-1, -1), (-1, 0), (-1, 1), (0, -1), (0, 0), (0, 1),
          (1, -1), (1, 0), (1, 1)]
HOST_SLOTS = [s for s in SLOT33 if s != (0, 0)]   # 8 real gate slots
DEV_SLOT = [0, 1, 2, 3, 5, 6, 7, 8]               # device slot of HOST_SLOTS[i]
# matmul emission order: center group (identity) first, then da=-1, da=+1
MM_ORDER = [3, 4, 5, 0, 1, 2, 6, 7, 8]
SMI = {0: 0, 1: 0, 2: 0, 3: 1, 4: 1, 5: 1, 6: 2, 7: 2, 8: 2}

AXES = ["x", "y", "z"]


def _axis_d(axis, da, db):
    if axis == "x":
        return (da, db, 0)
    if axis == "y":
        return (da, 0, db)
    return (0, da, db)


# db free-dim stride per axis (axis z: da is also free with stride ZC)
DBU = {"x": ZC, "y": 1}


def _shift_full(f, dx, dy, dz):
    """Zero-padded shift: out[x,y,z] = f[x+dx, y+dy, z+dz]."""
    o = np.zeros_like(f)
    tx0, tx1 = max(0, -dx), min(X, X - dx)
    ty0, ty1 = max(0, -dy), min(Y, Y - dy)
    tz0, tz1 = max(0, -dz), min(Z, Z - dz)
    o[tx0:tx1, ty0:ty1, tz0:tz1] = f[tx0 + dx:tx1 + dx, ty0 + dy:ty1 + dy,
                                     tz0 + dz:tz1 + dz]
    return o


def _slab(f, x0):
    """Full field [X,Y,Z] -> core slab [P, FD] (f32)."""
    pf = np.zeros((S, Y + 8, Z), dtype=np.float32)
    r0_, r1_ = x0 - M, x0 - M + S
    c0_, c1_ = max(0, r0_), min(X, r1_)
    pf[c0_ - r0_:c1_ - r0_, 2:Y + 2, :] = f[c0_:c1_]
    blocks = [pf[:, b * YT:b * YT + YC, :] for b in range(NYB)]
    return np.concatenate(blocks, axis=0).reshape(P, FD)


_COMPILED = None
_LAST_RESULTS = None


def _build_program():
    import concourse.bacc as bacc
    import concourse.mybir as mybir
    import concourse.tile as tile

    f32 = mybir.dt.float32
    bf16 = mybir.dt.bfloat16
    i16 = mybir.dt.int16
    MULT = mybir.AluOpType.mult
    AND = mybir.AluOpType.bitwise_and
    COPY = mybir.ActivationFunctionType.Copy

    nc = bacc.Bacc("TRN2", target_bir_lowering=False, debug=False,
                   num_devices=NCORES, dynamic_dma_scratch_size=2048)

    for val in (-EPS, EPS):
        ct = nc.alloc_sbuf_tensor(f"const-f32-{val}", [128, 1], f32)
        nc.gpsimd.memset(ct.ap(), val)
        nc.const_aps.aps[(f32, val)] = ct.ap()

    # ---- DRAM I/O ----
    g_in = {a: nc.dram_tensor(f"g_{a}", [P, 8, FD], bf16,
                              kind="ExternalInput").ap()
            for a in ("x", "y", "z")}
    gux_in = nc.dram_tensor("gux", [P, 6, FD], bf16,
                            kind="ExternalInput").ap()
    r0_in = nc.dram_tensor("r0", [P, FD], f32, kind="ExternalInput").ap()
    rc0_in = nc.dram_tensor("rc0", [P, SLOTF], bf16,
                            kind="ExternalInput").ap()
    rm0_in = nc.dram_tensor("rm0", [P, SLOTF], bf16,
                            kind="ExternalInput").ap()
    rp0_in = nc.dram_tensor("rp0", [P, SLOTF], bf16,
                            kind="ExternalInput").ap()
    shm_in = nc.dram_tensor("shm", [128, 3, 128], bf16,
                            kind="ExternalInput").ap()
    rout = nc.dram_tensor("rout", [P, FD], f32, kind="ExternalOutput").ap()

    with tile.TileContext(nc) as tc:
        with tc.tile_pool(name="stat", bufs=1) as st, \
             tc.tile_pool(name="wk", bufs=1) as wk, \
             tc.tile_pool(name="fin", bufs=3) as fin, \
             tc.tile_pool(name="psum", bufs=2, space="PSUM") as pp, \
             tc.tile_pool(name="psprepA", bufs=1, space="PSUM") as pqa, \
             tc.tile_pool(name="psprepB", bufs=2, space="PSUM") as pqb:

            # ---- static tiles ----
            t_g = {a: st.tile([P, 9, FD], bf16, tag=f"g{a}", name=f"t_g{a}")
                   for a in ("x", "y")}
            t_gz = [st.tile([P, 9, 512], bf16, tag=f"gz{i}", name=f"t_gz{i}")
                    for i in range(3)]
            t_nsz = st.tile([P, FD], bf16, tag="nsz", name="t_nsz")
            t_c1 = {a: st.tile([P, FD], bf16, tag=f"c1{a}", name=f"t_c1{a}")
                    for a in AXES}
            t_r = st.tile([P, FD], f32, tag="r", name="t_r")
            t_rs = st.tile([P, 3, SLOTF], bf16, tag="rs", name="t_rs")
            t_shm = st.tile([128, 3, 128], bf16, tag="shm", name="t_shm")
            t_p = [st.tile([P, 9, 512], bf16, tag=f"p{i}", name=f"t_p{i}")
                   for i in range(2)]
            t_tt = st.tile([P, 6, 512], bf16, tag="tt6", name="t_tt")

            APc = type(t_rs[:])
            rs_ap = t_rs[:]
            rs_pd = list(rs_ap.ap[0])
            rs_base = rs_ap.offset

            def win_rc(dbu, c0, cw):
                # all 9 slots on rc: [P, 3(da: routed, stride 0),
                #                     3(db win), cw]
                off = rs_base + SLOTF + GUARD + c0 - dbu
                return APc(rs_ap.tensor, off,
                           [rs_pd, [0, 3], [dbu, 3], [1, cw]])

            def win_z(c0, cw):
                # [P, 3(dy win), 3(dz win), cw] on rc
                off = rs_base + SLOTF + GUARD + c0 - ZC - 1
                return APc(rs_ap.tensor, off,
                           [rs_pd, [ZC, 3], [1, 3], [1, cw]])

            def win_s1(u, dbu, c0, cw):
                # step 1 group u: [P, 3(db win), cw] on host buffer u
                off = rs_base + u * SLOTF + GUARD + c0 - dbu
                return APc(rs_ap.tensor, off,
                           [rs_pd, [dbu, 3], [1, cw]])

            # ---- init ----
            nc.sync.dma_start(out=t_shm[:], in_=shm_in[:])
            nc.sync.dma_start(out=t_rs[:, 1, :], in_=rc0_in[:])
            nc.sync.dma_start(out=t_rs[:, 0, :], in_=rm0_in[:])
            nc.sync.dma_start(out=t_rs[:, 2, :], in_=rp0_in[:])
            nc.sync.dma_start(out=t_r[:], in_=r0_in[:])

            def load_resident(a):
                for ci in range(NCH):
                    c0, cw = CHUNKS[ci]
                    csl = slice(c0, c0 + cw)
                    nc.scalar.dma_start(out=t_g[a][:, 0:4, csl],
                                        in_=g_in[a][:, 0:4, csl])
                    nc.scalar.dma_start(out=t_g[a][:, 5:9, csl],
                                        in_=g_in[a][:, 4:8, csl])

            preps = {}   # (a, ci) -> (psA, psS) live PSUM tiles

            def prep_s1(a, ci, gsrc, smi_of, pq):
                """Gate-normalization sums for chunk ci of axis a.
                gsrc: AP [P, 9, cw] (slots 0-3, 5-9 hold gates)."""
                c0, cw = CHUNKS[ci]
                psA = pq.tile([P, 512], f32, tag="psA", name="psA")
                psS = pq.tile([P, 512], f32, tag="psS", name="psS")
                for h, sl in ((0, slice(0, 4)), (1, slice(5, 9))):
                    tabs = wk.tile([P, 4, 512], bf16, tag="tabs", name="tabs")
                    nc.vector.tensor_scalar(
                        tabs[:, :, 0:cw].bitcast(i16),
                        gsrc[:, sl, :].bitcast(i16),
                        0x7FFF, None, AND)
                    for j in range(4):
                        s = (0, 1, 2, 3)[j] if h == 0 else (5, 6, 7, 8)[j]
                        nc.tensor.matmul(psA[:, 0:cw],
                                         t_shm[0:P, smi_of(s), 0:P],
                                         tabs[:, j, 0:cw],
                                         start=(h == 0 and j == 0),
                                         stop=(h == 1 and j == 3))
                for j, s in enumerate(DEV_SLOT):
                    nc.tensor.matmul(psS[:, 0:cw],
                                     t_shm[0:P, smi_of(s), 0:P],
                                     gsrc[:, s, :],
                                     start=(j == 0), stop=(j == 7))
                preps[(a, ci)] = (psA, psS)

            def prep_s2(a, ci, ns_dst):
                """Normalize: c1 and -S from the stage-1 sums."""
                c0, cw = CHUNKS[ci]
                csl = slice(c0, c0 + cw)
                psA, psS = preps.pop((a, ci))
                # c1 = 1/max(A, eps): Relu(A-eps)+eps is exact in f32
                tA = wk.tile([P, 512], f32, tag="tA", name="tA")
                nc.scalar.activation(tA[:, 0:cw], psA[:, 0:cw],
                                     mybir.ActivationFunctionType.Relu,
                                     bias=-EPS, scale=1.0)
                nc.scalar.activation(tA[:, 0:cw], tA[:, 0:cw],
                                     mybir.ActivationFunctionType.Identity,
                                     bias=EPS, scale=1.0)
                nc.vector.reciprocal_approx_fast(tA[:, 0:cw], tA[:, 0:cw])
                nc.scalar.activation(t_c1[a][:, csl], tA[:, 0:cw], COPY)
                # nS = -S (bf16)
                nc.scalar.activation(ns_dst, psS[:, 0:cw], COPY, scale=-1.0)

            def prep_chunk(a, ci, gsrc, ns_dst, smi_of):
                prep_s1(a, ci, gsrc, smi_of, pqa)
                prep_s2(a, ci, ns_dst)

            gchunk = [0]   # global chunk counter: t_p buffer parity

            def emit_step(step, a, pre_chunk=None, post_chunk=None,
                          zres=False):
                """One propagation step. step in 1..9."""
                zstep = a == "z"
                first = step == 1
                if step in (4, 7):
                    # y-ghost col refresh in rc space (blocks overlap by 2
                    # cols each side; y-touching steps 1,3 / 4,6 / 7,9
                    # consume one col per side between refreshes)
                    gi = GUARD
                    nc.sync.dma_start(
                        out=t_rs[S:P, 1, gi:gi + 2 * ZC],
                        in_=t_rs[0:P - S, 1,
                                 gi + YT * ZC:gi + YT * ZC + 2 * ZC])
                    nc.gpsimd.dma_start(
                        out=t_rs[0:P - S, 1, gi + FD - 2 * ZC:gi + FD],
                        in_=t_rs[S:P, 1, gi + 2 * ZC:gi + 4 * ZC])
                corder = list(range(NCH))

                def rc_update(ci):
                    c0_, cw_ = CHUNKS[ci]
                    rcc = t_rs[:, 1, GUARD + c0_:GUARD + c0_ + cw_]
                    nc.vector.tensor_tensor(
                        out=rcc, in0=t_tt[:, ci, 0:cw_], in1=rcc,
                        op=mybir.AluOpType.add)

                for pos, ci in enumerate(corder):
                    c0, cw = CHUNKS[ci]
                    csl = slice(c0, c0 + cw)
                    gchunk[0] += 1
                    if pre_chunk is not None:
                        pre_chunk(ci)
                    if zstep and not zres:
                        zbuf = pos % 3
                        if pos >= 3:   # pos 0,1,2 were prefetched
                            nc.scalar.dma_start(out=t_gz[zbuf][:, 0:4, 0:cw],
                                                in_=g_in["z"][:, 0:4, csl])
                            nc.scalar.dma_start(out=t_gz[zbuf][:, 5:9, 0:cw],
                                                in_=g_in["z"][:, 4:8, csl])
                        if step == 3:
                            prep_chunk("z", ci, t_gz[zbuf][:, :, 0:cw],
                                       t_gz[zbuf][:, 4, 0:cw], lambda s: 1)
                            nc.vector.tensor_scalar(
                                t_nsz[:, csl], t_gz[zbuf][:, 4, 0:cw],
                                1.0, None, MULT)
                        else:
                            nc.vector.tensor_scalar(
                                t_gz[zbuf][:, 4, 0:cw], t_nsz[:, csl],
                                1.0, None, MULT)
                    buf = gchunk[0] % 2
                    if zstep:
                        zin0 = (t_g["x"][:, :, csl] if zres
                                else t_gz[zbuf][:, :, 0:cw])
                        nc.vector.tensor_tensor(
                            out=t_p[buf][:, :, 0:cw]
                            .rearrange("p (u v) f -> p u v f", u=3),
                            in0=zin0.rearrange("p (u v) f -> p u v f", u=3),
                            in1=win_z(c0, cw), op=MULT)
                    elif first:
                        # stream unbaked da=+-1 groups; center from resident
                        zbuf = pos % 2
                        nc.sync.dma_start(out=t_gz[zbuf][:, 0:3, 0:cw],
                                          in_=gux_in[:, 0:3, csl])
                        nc.sync.dma_start(out=t_gz[zbuf][:, 6:9, 0:cw],
                                          in_=gux_in[:, 3:6, csl])
                        for u, src in ((0, t_gz[zbuf]), (1, t_g[a]),
                                       (2, t_gz[zbuf])):
                            if u == 1:
                                in0 = src[:, 3:6, csl]
                            else:
                                in0 = src[:, 3 * u:3 * u + 3, 0:cw]
                            nc.vector.tensor_tensor(
                                out=t_p[buf][:, 3 * u:3 * u + 3, 0:cw],
                                in0=in0,
                                in1=win_s1(u, DBU[a], c0, cw), op=MULT)
                    else:
                        nc.vector.tensor_tensor(
                            out=t_p[buf][:, :, 0:cw]
                            .rearrange("p (u v) f -> p u v f", u=3),
                            in0=t_g[a][:, :, csl]
                            .rearrange("p (u v) f -> p u v f", u=3),
                            in1=win_rc(DBU[a], c0, cw), op=MULT)
                    tps = pp.tile([P, 512], f32, tag="tps", name="tps")
                    for mi, s in enumerate(MM_ORDER):
                        smi = 1 if (first or zstep) else SMI[s]
                        nc.tensor.matmul(tps[:, 0:cw], t_shm[0:P, smi, 0:P],
                                         t_p[buf][:, s, 0:cw],
                                         start=(mi == 0), stop=(mi == 8))
                    # combine: psb = bf16(psum) [Act]; t = c1*psb [Pool];
                    # rc += t [DVE, the propagating bf16 state];
                    # r_f32 += t [Pool, off the critical path, output only]
                    psb = fin.tile([P, 512], bf16, tag="psb", name="psb")
                    nc.scalar.activation(psb[:, 0:cw], tps[:, 0:cw], COPY)
                    tt = t_tt[:, ci, :]
                    nc.gpsimd.tensor_tensor(
                        out=tt[:, 0:cw], in0=psb[:, 0:cw],
                        in1=t_c1[a][:, csl], op=MULT)
                    nc.gpsimd.tensor_tensor(
                        out=t_r[:, csl], in0=tt[:, 0:cw],
                        in1=t_r[:, csl], op=mybir.AluOpType.add)
                    if step == 9:
                        nc.sync.dma_start(out=rout[:, csl], in_=t_r[:, csl])
                    if post_chunk is not None:
                        post_chunk(ci)
                    # rc update, two positions behind: legal once both
                    # f-neighbor products are emitted (DVE runs in order, and
                    # a product's window reads the neighbor's edge cols); the
                    # extra position gives the Pool combine chain time to
                    # deliver tt without head-of-line blocking the DVE.
                    if pos >= 2:
                        rc_update(corder[pos - 2])
                rc_update(corder[-2])
                rc_update(corder[-1])

            # ---- schedule ----
            def prep_of(a):
                def f(ci):
                    c0, cw = CHUNKS[ci]
                    csl = slice(c0, c0 + cw)
                    prep_chunk(a, ci, t_g[a][:, :, csl],
                               t_g[a][:, 4, csl], lambda s: SMI[s])
                return f

            def z_prefetch(zstep):
                for pos, ci in enumerate((0, 1, 2)):
                    c0, cw = CHUNKS[ci]
                    csl = slice(c0, c0 + cw)
                    nc.scalar.dma_start(out=t_gz[pos][:, 0:4, 0:cw],
                                        in_=g_in["z"][:, 0:4, csl])
                    nc.scalar.dma_start(out=t_gz[pos][:, 5:9, 0:cw],
                                        in_=g_in["z"][:, 4:8, csl])

            def y_s1(ci):
                c0, cw = CHUNKS[ci]
                csl = slice(c0, c0 + cw)
                prep_s1("y", ci, t_g["y"][:, :, csl], lambda s: SMI[s], pqb)

            def y_s2(ci):
                c0, cw = CHUNKS[ci]
                prep_s2("y", ci, t_g["y"][:, 4, c0:c0 + cw])

            def step1_post(ci):
                # software-pipelined prep-y: loads now, sums one chunk
                # behind, normalize two behind, so the long dependency
                # chains never head-of-line-block the in-order DVE.
                c0, cw = CHUNKS[ci]
                csl = slice(c0, c0 + cw)
                nc.scalar.dma_start(out=t_g["y"][:, 0:4, csl],
                                    in_=g_in["y"][:, 0:4, csl])
                nc.scalar.dma_start(out=t_g["y"][:, 5:9, csl],
                                    in_=g_in["y"][:, 4:8, csl])
                if ci >= 1:
                    y_s1(ci - 1)
                if ci >= 2:
                    y_s2(ci - 2)

            load_resident("x")
            # prep-x feeds step 1's combine; prep-y rides along to keep the
            # PE busy during step 1's product stalls.
            emit_step(1, "x", pre_chunk=prep_of("x"), post_chunk=step1_post)
            y_s1(NCH - 1)
            y_s2(NCH - 2)
            y_s2(NCH - 1)
            step = 1
            for it in range(PROP_TIME):
                for a in AXES:
                    if it == 0 and a == "x":
                        continue
                    step += 1
                    if a == "z" and step != 9:
                        z_prefetch(step)
                    emit_step(step, a, zres=(step == 9))
                    if step == 7:
                        # t_g["x"] is dead after step 7: make step 9's z
                        # gates fully resident there during step 8
                        for c0, cw in CHUNKS:
                            csl = slice(c0, c0 + cw)
                            nc.scalar.dma_start(out=t_g["x"][:, 0:4, csl],
                                                in_=g_in["z"][:, 0:4, csl])
                            nc.scalar.dma_start(out=t_g["x"][:, 5:9, csl],
                                                in_=g_in["z"][:, 4:8, csl])
                            nc.vector.tensor_scalar(
                                t_g["x"][:, 4, csl], t_nsz[:, csl],
                                1.0, None, MULT)

    nc.compile()
    return nc


def _prep_inputs(guidance, blur):
    """Host-side swizzle: build per-core input dicts (layout only)."""
    guidance = np.asarray(guidance, dtype=np.float32)
    blur = np.asarray(blur, dtype=np.float32)[0]  # [X,Y,Z]
    x0s = [c * W for c in range(NCORES)]

    in_maps = [dict() for _ in range(NCORES)]

    # shift matrices sm[p, g, q]: route product at partition p=q+da -> q.
    # g=0 (da=-1): q=p+1 ; g=1: q=p ; g=2 (da=+1): q=p-1
    sm = np.zeros((128, 3, 128), dtype=BF)
    for q in range(P):
        if q - 1 >= 0:
            sm[q - 1, 0, q] = 1.0
        sm[q, 1, q] = 1.0
        if q + 1 < P:
            sm[q + 1, 2, q] = 1.0
    for c in range(NCORES):
        in_maps[c]["shm"] = sm

    for ai, a in enumerate(AXES):
        base = 8 * ai
        shifted = np.empty((8, X, Y, Z), dtype=np.float32)
        for si, (da, db) in enumerate(HOST_SLOTS):
            k = DLIST.index((da, db))
            dx, dy, dz = _axis_d(a, da, db)
            shifted[si] = _shift_full(guidance[base + k], dx, dy, dz)
        for c in range(NCORES):
            ga = np.empty((P, 8, FD), dtype=np.float32)
            for si, (da, db) in enumerate(HOST_SLOTS):
                bake = da if a in ("x", "y") else 0
                ga[:, si] = _slab(shifted[si], x0s[c] - bake)
            in_maps[c][f"g_{a}"] = ga.astype(BF)
            if a == "x":
                gu = np.empty((P, 6, FD), dtype=np.float32)
                for j, si in enumerate((0, 1, 2, 5, 6, 7)):
                    gu[:, j] = _slab(shifted[si], x0s[c])
                in_maps[c]["gux"] = gu.astype(BF)

    for c in range(NCORES):
        in_maps[c]["r0"] = _slab(blur, x0s[c]).astype(np.float32)
        for name, dx in (("rc0", 0), ("rm0", -1), ("rp0", 1)):
            sl = np.zeros((P, SLOTF), dtype=BF)
            sl[:, GUARD:GUARD + FD] = _slab(blur, x0s[c] + dx).astype(BF)
            in_maps[c][name] = sl

    return in_maps


def _unswizzle(results):
    out = np.empty((1, X, Y, Z), dtype=np.float32)
    for c in range(NCORES):
        r = results[c]["rout"].reshape(P, YC, ZC)
        x0 = c * W
        for b in range(NYB):
            ys = b * YT
            ye = min(Y, ys + YT)
            out[0, x0:x0 + W, ys:ye, :] = \
                r[b * S + M: b * S + M + W, 2:2 + (ye - ys), :]
    return out


def kernel(guidance, blur):
    global _COMPILED, _LAST_RESULTS
    from concourse import bass_utils
    if _COMPILED is None:
        _COMPILED = _build_program()
    nc = _COMPILED
    in_maps = _prep_inputs(guidance, blur)
    res = bass_utils.run_bass_kernel_spmd(nc, in_maps,
                                          core_ids=list(range(NCORES)))
    _LAST_RESULTS = res
    return _unswizzle(res.results)


# revision 52
# speedup vs baseline: 1.0134x; 1.0134x over previous
"""Affinity-propagation (CSPN-3D) Trainium2 kernel, v3.

Problem: guidance [24,256,256,32] f32, blur [1,256,256,32] f32.
3 iterations of (x-plane, y-plane, z-plane) 8-neighbor gated propagation:

  out(q) = r(q) + c1(q) * [ sum_k G_k(q+d_k) * r(q+d_k) - S(q) * r(q) ]
  A(q) = sum_k |G_k(q+d_k)|, S(q) = sum_k G_k(q+d_k), c1 = 1/max(A,eps)

Sharding: 8 cores, X sharded 32 rows/core, ghost margin 5 rows each side;
step 1 consumes no margin (host supplies +-1-x-shifted blur slabs and
unbaked x gates), the remaining 5 x-crossing steps consume 1 each.

Per-core layout: partitions p = yb*42 + xl (3 y-blocks x 42 x-rows = 126);
free f = ylocal*32 + z, ylocal in [0,88) (86-wide y third + 1 overlap col
each side), z in [0,32) unpadded (z boundary handled by zero gates).
FD = 2816, chunked [512 x 5, 256].

Gates are host-pre-shifted by their full 3D offset and, for the x/y axes,
additionally "baked" by -da along partitions so products are computed
against the partition-local rc copy; a PE matmul with a +-1-shift
stationary routes each da group back while accumulating all 9 slots
(8 gates + a -S center slot) into PSUM. Per step:
 - Act: rc = bf16(r) into a guarded window buffer; psum -> bf16 drain.
 - DVE: one 9-slot product instruction per chunk (windowed 4-dim AP).
 - PE: 9 matmuls (3 stationaries) accumulate slots into PSUM f32.
 - Pool: t = c1*psum_bf16 ; r += t.
A/S/c1 are computed on device: |g| via 4x-mode bitmask (DVE), slot sums
via the same PE routing, 1/A via DVE reciprocal.
"""

import numpy as np
import ml_dtypes

BF = ml_dtypes.bfloat16

X = Y = 256
Z = 32
NCORES = 8
W = X // NCORES          # 32 interior x rows per core
M = 5                    # ghost margin rows
S = W + 2 * M            # 42 slab rows
NYB = 3                  # y thirds
YT = 86                  # y third width
YC = YT + 4              # y cols incl 2 overlap each side
ZC = Z                   # z cols, unpadded
FD = YC * ZC             # 2880
P = NYB * S              # 126 partitions
CHUNKS = [(0, 512), (512, 512), (1024, 512), (1536, 512),
          (2048, 512), (2560, 320)]
NCH = len(CHUNKS)
GUARD = 34               # window guard (max offset ZC+1=33)
SLOTF = GUARD + FD + GUARD   # 2884
PROP_TIME = 3
EPS = 1e-30

# k -> (dH, dW) neighbor offsets, matching reference PADS
DLIST = [(1, 1), (1, 0), (1, -1), (0, 1), (0, -1), (-1, 1), (-1, 0), (-1, -1)]
# 3x3 slot enumeration (da, db), row-major; center (0,0) is the nS slot.
SLOT33 = [(-1, -1), (-1, 0), (-1, 1), (0, -1), (0, 0), (0, 1),
          (1, -1), (1, 0), (1, 1)]
HOST_SLOTS = [s for s in SLOT33 if s != (0, 0)]   # 8 real gate slots
DEV_SLOT = [0, 1, 2, 3, 5, 6, 7, 8]               # device slot of HOST_SLOTS[i]
# matmul emission order: center group (identity) first, then da=-1, da=+1
MM_ORDER = [3, 4, 5, 0, 1, 2, 6, 7, 8]
SMI = {0: 0, 1: 0, 2: 0, 3: 1, 4: 1, 5: 1, 6: 2, 7: 2, 8: 2}

AXES = ["x", "y", "z"]


def _axis_d(axis, da, db):
    if axis == "x":
        return (da, db, 0)
    if axis == "y":
        return (da, 0, db)
    return (0, da, db)


# db free-dim stride per axis (axis z: da is also free with stride ZC)
DBU = {"x": ZC, "y": 1}


def _shift_full(f, dx, dy, dz):
    """Zero-padded shift: out[x,y,z] = f[x+dx, y+dy, z+dz]."""
    o = np.zeros_like(f)
    tx0, tx1 = max(0, -dx), min(X, X - dx)
    ty0, ty1 = max(0, -dy), min(Y, Y - dy)
    tz0, tz1 = max(0, -dz), min(Z, Z - dz)
    o[tx0:tx1, ty0:ty1, tz0:tz1] = f[tx0 + dx:tx1 + dx, ty0 + dy:ty1 + dy,
                                     tz0 + dz:tz1 + dz]
    return o


def _slab(f, x0):
    """Full field [X,Y,Z] -> core slab [P, FD] (f32)."""
    pf = np.zeros((S, Y + 8, Z), dtype=np.float32)
    r0_, r1_ = x0 - M, x0 - M + S
    c0_, c1_ = max(0, r0_), min(X, r1_)
    pf[c0_ - r0_:c1_ - r0_, 2:Y + 2, :] = f[c0_:c1_]
    blocks = [pf[:, b * YT:b * YT + YC, :] for b in range(NYB)]
    return np.concatenate(blocks, axis=0).reshape(P, FD)


_COMPILED = None
_LAST_RESULTS = None


def _build_program():
    import concourse.bacc as bacc
    import concourse.mybir as mybir
    import concourse.tile as tile

    f32 = mybir.dt.float32
    bf16 = mybir.dt.bfloat16
    i16 = mybir.dt.int16
    MULT = mybir.AluOpType.mult
    AND = mybir.AluOpType.bitwise_and
    COPY = mybir.ActivationFunctionType.Copy

    nc = bacc.Bacc("TRN2", target_bir_lowering=False, debug=False,
                   num_devices=NCORES, dynamic_dma_scratch_size=2048)

    for val in (-EPS, EPS):
        ct = nc.alloc_sbuf_tensor(f"const-f32-{val}", [128, 1], f32)
        nc.gpsimd.memset(ct.ap(), val)
        nc.const_aps.aps[(f32, val)] = ct.ap()

    # ---- DRAM I/O ----
    g_in = {a: nc.dram_tensor(f"g_{a}", [P, 8, FD], bf16,
                              kind="ExternalInput").ap()
            for a in ("x", "y", "z")}
    gux_in = nc.dram_tensor("gux", [P, 6, FD], bf16,
                            kind="ExternalInput").ap()
    r0_in = nc.dram_tensor("r0", [P, FD], f32, kind="ExternalInput").ap()
    rm0_in = nc.dram_tensor("rm0", [P, SLOTF], bf16,
                            kind="ExternalInput").ap()
    rp0_in = nc.dram_tensor("rp0", [P, SLOTF], bf16,
                            kind="ExternalInput").ap()
    shm_in = nc.dram_tensor("shm", [128, 3, 128], bf16,
                            kind="ExternalInput").ap()
    rout = nc.dram_tensor("rout", [P, FD], f32, kind="ExternalOutput").ap()

    with tile.TileContext(nc) as tc:
        with tc.tile_pool(name="stat", bufs=1) as st, \
             tc.tile_pool(name="wk", bufs=1) as wk, \
             tc.tile_pool(name="fin", bufs=3) as fin, \
             tc.tile_pool(name="psum", bufs=2, space="PSUM") as pp, \
             tc.tile_pool(name="psprepA", bufs=1, space="PSUM") as pqa, \
             tc.tile_pool(name="psprepB", bufs=2, space="PSUM") as pqb:

            # ---- static tiles ----
            t_g = {a: st.tile([P, 9, FD], bf16, tag=f"g{a}", name=f"t_g{a}")
                   for a in ("x", "y")}
            t_gz = [st.tile([P, 9, 512], bf16, tag=f"gz{i}", name=f"t_gz{i}")
                    for i in range(3)]
            t_nsz = st.tile([P, FD], bf16, tag="nsz", name="t_nsz")
            t_c1 = {a: st.tile([P, FD], bf16, tag=f"c1{a}", name=f"t_c1{a}")
                    for a in AXES}
            t_r = st.tile([P, FD], f32, tag="r", name="t_r")
            t_rs = st.tile([P, 3, SLOTF], bf16, tag="rs", name="t_rs")
            t_shm = st.tile([128, 3, 128], bf16, tag="shm", name="t_shm")
            t_p = [st.tile([P, 9, 512], bf16, tag=f"p{i}", name=f"t_p{i}")
                   for i in range(2)]
            t_tt = st.tile([P, 6, 512], bf16, tag="tt6", name="t_tt")

            APc = type(t_rs[:])
            rs_ap = t_rs[:]
            rs_pd = list(rs_ap.ap[0])
            rs_base = rs_ap.offset

            def win_rc(dbu, c0, cw):
                # all 9 slots on rc: [P, 3(da: routed, stride 0),
                #                     3(db win), cw]
                off = rs_base + SLOTF + GUARD + c0 - dbu
                return APc(rs_ap.tensor, off,
                           [rs_pd, [0, 3], [dbu, 3], [1, cw]])

            def win_z(c0, cw):
                # [P, 3(dy win), 3(dz win), cw] on rc
                off = rs_base + SLOTF + GUARD + c0 - ZC - 1
                return APc(rs_ap.tensor, off,
                           [rs_pd, [ZC, 3], [1, 3], [1, cw]])

            def win_s1(u, dbu, c0, cw):
                # step 1 group u: [P, 3(db win), cw] on host buffer u
                off = rs_base + u * SLOTF + GUARD + c0 - dbu
                return APc(rs_ap.tensor, off,
                           [rs_pd, [dbu, 3], [1, cw]])

            # ---- init ----
            nc.sync.dma_start(out=t_shm[:], in_=shm_in[:])
            nc.gpsimd.memset(t_rs[:], 0.0)
            nc.sync.dma_start(out=t_r[:], in_=r0_in[:])
            nc.sync.dma_start(out=t_rs[:, 0, :], in_=rm0_in[:])
            nc.sync.dma_start(out=t_rs[:, 2, :], in_=rp0_in[:])

            def load_resident(a):
                for ci in range(NCH):
                    c0, cw = CHUNKS[ci]
                    csl = slice(c0, c0 + cw)
                    nc.scalar.dma_start(out=t_g[a][:, 0:4, csl],
                                        in_=g_in[a][:, 0:4, csl])
                    nc.scalar.dma_start(out=t_g[a][:, 5:9, csl],
                                        in_=g_in[a][:, 4:8, csl])

            preps = {}   # (a, ci) -> (psA, psS) live PSUM tiles

            def prep_s1(a, ci, gsrc, smi_of, pq):
                """Gate-normalization sums for chunk ci of axis a.
                gsrc: AP [P, 9, cw] (slots 0-3, 5-9 hold gates)."""
                c0, cw = CHUNKS[ci]
                psA = pq.tile([P, 512], f32, tag="psA", name="psA")
                psS = pq.tile([P, 512], f32, tag="psS", name="psS")
                for h, sl in ((0, slice(0, 4)), (1, slice(5, 9))):
                    tabs = wk.tile([P, 4, 512], bf16, tag="tabs", name="tabs")
                    nc.vector.tensor_scalar(
                        tabs[:, :, 0:cw].bitcast(i16),
                        gsrc[:, sl, :].bitcast(i16),
                        0x7FFF, None, AND)
                    for j in range(4):
                        s = (0, 1, 2, 3)[j] if h == 0 else (5, 6, 7, 8)[j]
                        nc.tensor.matmul(psA[:, 0:cw],
                                         t_shm[0:P, smi_of(s), 0:P],
                                         tabs[:, j, 0:cw],
                                         start=(h == 0 and j == 0),
                                         stop=(h == 1 and j == 3))
                for j, s in enumerate(DEV_SLOT):
                    nc.tensor.matmul(psS[:, 0:cw],
                                     t_shm[0:P, smi_of(s), 0:P],
                                     gsrc[:, s, :],
                                     start=(j == 0), stop=(j == 7))
                preps[(a, ci)] = (psA, psS)

            def prep_s2(a, ci, ns_dst):
                """Normalize: c1 and -S from the stage-1 sums."""
                c0, cw = CHUNKS[ci]
                csl = slice(c0, c0 + cw)
                psA, psS = preps.pop((a, ci))
                # c1 = 1/max(A, eps): Relu(A-eps)+eps is exact in f32
                tA = wk.tile([P, 512], f32, tag="tA", name="tA")
                nc.scalar.activation(tA[:, 0:cw], psA[:, 0:cw],
                                     mybir.ActivationFunctionType.Relu,
                                     bias=-EPS, scale=1.0)
                nc.scalar.activation(tA[:, 0:cw], tA[:, 0:cw],
                                     mybir.ActivationFunctionType.Identity,
                                     bias=EPS, scale=1.0)
                nc.vector.reciprocal_approx_fast(tA[:, 0:cw], tA[:, 0:cw])
                nc.scalar.activation(t_c1[a][:, csl], tA[:, 0:cw], COPY)
                # nS = -S (bf16)
                nc.scalar.activation(ns_dst, psS[:, 0:cw], COPY, scale=-1.0)

            def prep_chunk(a, ci, gsrc, ns_dst, smi_of):
                prep_s1(a, ci, gsrc, smi_of, pqa)
                prep_s2(a, ci, ns_dst)

            gchunk = [0]   # global chunk counter: t_p buffer parity

            def emit_step(step, a, pre_chunk=None, post_chunk=None,
                          zres=False):
                """One propagation step. step in 1..9."""
                zstep = a == "z"
                first = step == 1
                if step in (4, 7):
                    # y-ghost col refresh in rc space (blocks overlap by 2
                    # cols each side; y-touching steps 1,3 / 4,6 / 7,9
                    # consume one col per side between refreshes)
                    gi = GUARD
                    nc.sync.dma_start(
                        out=t_rs[S:P, 1, gi:gi + 2 * ZC],
                        in_=t_rs[0:P - S, 1,
                                 gi + YT * ZC:gi + YT * ZC + 2 * ZC])
                    nc.gpsimd.dma_start(
                        out=t_rs[0:P - S, 1, gi + FD - 2 * ZC:gi + FD],
                        in_=t_rs[S:P, 1, gi + 2 * ZC:gi + 4 * ZC])
                corder = list(range(NCH))

                def rc_update(ci):
                    c0_, cw_ = CHUNKS[ci]
                    rcc = t_rs[:, 1, GUARD + c0_:GUARD + c0_ + cw_]
                    nc.vector.tensor_tensor(
                        out=rcc, in0=t_tt[:, ci, 0:cw_], in1=rcc,
                        op=mybir.AluOpType.add)

                for pos, ci in enumerate(corder):
                    c0, cw = CHUNKS[ci]
                    csl = slice(c0, c0 + cw)
                    gchunk[0] += 1
                    if pre_chunk is not None:
                        pre_chunk(ci)
                    if zstep and not zres:
                        zbuf = pos % 3
                        if pos >= 3:   # pos 0,1,2 were prefetched
                            nc.scalar.dma_start(out=t_gz[zbuf][:, 0:4, 0:cw],
                                                in_=g_in["z"][:, 0:4, csl])
                            nc.scalar.dma_start(out=t_gz[zbuf][:, 5:9, 0:cw],
                                                in_=g_in["z"][:, 4:8, csl])
                        if step == 3:
                            prep_chunk("z", ci, t_gz[zbuf][:, :, 0:cw],
                                       t_gz[zbuf][:, 4, 0:cw], lambda s: 1)
                            nc.vector.tensor_scalar(
                                t_nsz[:, csl], t_gz[zbuf][:, 4, 0:cw],
                                1.0, None, MULT)
                        else:
                            nc.vector.tensor_scalar(
                                t_gz[zbuf][:, 4, 0:cw], t_nsz[:, csl],
                                1.0, None, MULT)
                    buf = gchunk[0] % 2
                    if zstep:
                        zin0 = (t_g["x"][:, :, csl] if zres
                                else t_gz[zbuf][:, :, 0:cw])
                        nc.vector.tensor_tensor(
                            out=t_p[buf][:, :, 0:cw]
                            .rearrange("p (u v) f -> p u v f", u=3),
                            in0=zin0.rearrange("p (u v) f -> p u v f", u=3),
                            in1=win_z(c0, cw), op=MULT)
                    elif first:
                        # stream unbaked da=+-1 groups; center from resident
                        zbuf = pos % 2
                        nc.sync.dma_start(out=t_gz[zbuf][:, 0:3, 0:cw],
                                          in_=gux_in[:, 0:3, csl])
                        nc.sync.dma_start(out=t_gz[zbuf][:, 6:9, 0:cw],
                                          in_=gux_in[:, 3:6, csl])
                        for u, src in ((0, t_gz[zbuf]), (1, t_g[a]),
                                       (2, t_gz[zbuf])):
                            if u == 1:
                                in0 = src[:, 3:6, csl]
                            else:
                                in0 = src[:, 3 * u:3 * u + 3, 0:cw]
                            nc.vector.tensor_tensor(
                                out=t_p[buf][:, 3 * u:3 * u + 3, 0:cw],
                                in0=in0,
                                in1=win_s1(u, DBU[a], c0, cw), op=MULT)
                    else:
                        nc.vector.tensor_tensor(
                            out=t_p[buf][:, :, 0:cw]
                            .rearrange("p (u v) f -> p u v f", u=3),
                            in0=t_g[a][:, :, csl]
                            .rearrange("p (u v) f -> p u v f", u=3),
                            in1=win_rc(DBU[a], c0, cw), op=MULT)
                    tps = pp.tile([P, 512], f32, tag="tps", name="tps")
                    for mi, s in enumerate(MM_ORDER):
                        smi = 1 if (first or zstep) else SMI[s]
                        nc.tensor.matmul(tps[:, 0:cw], t_shm[0:P, smi, 0:P],
                                         t_p[buf][:, s, 0:cw],
                                         start=(mi == 0), stop=(mi == 8))
                    # combine: psb = bf16(psum) [Act]; t = c1*psb [Pool];
                    # rc += t [DVE, the propagating bf16 state];
                    # r_f32 += t [Pool, off the critical path, output only]
                    psb = fin.tile([P, 512], bf16, tag="psb", name="psb")
                    nc.scalar.activation(psb[:, 0:cw], tps[:, 0:cw], COPY)
                    tt = t_tt[:, ci, :]
                    nc.gpsimd.tensor_tensor(
                        out=tt[:, 0:cw], in0=psb[:, 0:cw],
                        in1=t_c1[a][:, csl], op=MULT)
                    nc.gpsimd.tensor_tensor(
                        out=t_r[:, csl], in0=tt[:, 0:cw],
                        in1=t_r[:, csl], op=mybir.AluOpType.add)
                    if post_chunk is not None:
                        post_chunk(ci)
                    # rc update, two positions behind: legal once both
                    # f-neighbor products are emitted (DVE runs in order, and
                    # a product's window reads the neighbor's edge cols); the
                    # extra position gives the Pool combine chain time to
                    # deliver tt without head-of-line blocking the DVE.
                    if pos >= 2:
                        rc_update(corder[pos - 2])
                rc_update(corder[-2])
                rc_update(corder[-1])

            # ---- schedule ----
            def prep_of(a):
                def f(ci):
                    c0, cw = CHUNKS[ci]
                    csl = slice(c0, c0 + cw)
                    prep_chunk(a, ci, t_g[a][:, :, csl],
                               t_g[a][:, 4, csl], lambda s: SMI[s])
                return f

            def z_prefetch(zstep):
                for pos, ci in enumerate((0, 1, 2)):
                    c0, cw = CHUNKS[ci]
                    csl = slice(c0, c0 + cw)
                    nc.scalar.dma_start(out=t_gz[pos][:, 0:4, 0:cw],
                                        in_=g_in["z"][:, 0:4, csl])
                    nc.scalar.dma_start(out=t_gz[pos][:, 5:9, 0:cw],
                                        in_=g_in["z"][:, 4:8, csl])

            def y_s1(ci):
                c0, cw = CHUNKS[ci]
                csl = slice(c0, c0 + cw)
                prep_s1("y", ci, t_g["y"][:, :, csl], lambda s: SMI[s], pqb)

            def y_s2(ci):
                c0, cw = CHUNKS[ci]
                prep_s2("y", ci, t_g["y"][:, 4, c0:c0 + cw])

            def step1_post(ci):
                # software-pipelined prep-y: loads now, sums one chunk
                # behind, normalize two behind, so the long dependency
                # chains never head-of-line-block the in-order DVE.
                c0, cw = CHUNKS[ci]
                csl = slice(c0, c0 + cw)
                nc.scalar.dma_start(out=t_g["y"][:, 0:4, csl],
                                    in_=g_in["y"][:, 0:4, csl])
                nc.scalar.dma_start(out=t_g["y"][:, 5:9, csl],
                                    in_=g_in["y"][:, 4:8, csl])
                if ci >= 1:
                    y_s1(ci - 1)
                if ci >= 2:
                    y_s2(ci - 2)

            load_resident("x")
            # prefix: rc = bf16(r0)
            for c0, cw in CHUNKS:
                nc.scalar.activation(
                    t_rs[:, 1, GUARD + c0:GUARD + c0 + cw],
                    t_r[:, c0:c0 + cw], COPY)
            # prep-x feeds step 1's combine; prep-y rides along to keep the
            # PE busy during step 1's product stalls.
            emit_step(1, "x", pre_chunk=prep_of("x"), post_chunk=step1_post)
            y_s1(NCH - 1)
            y_s2(NCH - 2)
            y_s2(NCH - 1)
            step = 1
            for it in range(PROP_TIME):
                for a in AXES:
                    if it == 0 and a == "x":
                        continue
                    step += 1
                    if a == "z" and step != 9:
                        z_prefetch(step)
                    emit_step(step, a, zres=(step == 9))
                    if step == 7:
                        # t_g["x"] is dead after step 7: make step 9's z
                        # gates fully resident there during step 8
                        for c0, cw in CHUNKS:
                            csl = slice(c0, c0 + cw)
                            nc.scalar.dma_start(out=t_g["x"][:, 0:4, csl],
                                                in_=g_in["z"][:, 0:4, csl])
                            nc.scalar.dma_start(out=t_g["x"][:, 5:9, csl],
                                                in_=g_in["z"][:, 4:8, csl])
                            nc.vector.tensor_scalar(
                                t_g["x"][:, 4, csl], t_nsz[:, csl],
                                1.0, None, MULT)

            nc.sync.dma_start(out=rout[:], in_=t_r[:])

    nc.compile()
    return nc


def _prep_inputs(guidance, blur):
    """Host-side swizzle: build per-core input dicts (layout only)."""
    guidance = np.asarray(guidance, dtype=np.float32)
    blur = np.asarray(blur, dtype=np.float32)[0]  # [X,Y,Z]
    x0s = [c * W for c in range(NCORES)]

    in_maps = [dict() for _ in range(NCORES)]

    # shift matrices sm[p, g, q]: route product at partition p=q+da -> q.
    # g=0 (da=-1): q=p+1 ; g=1: q=p ; g=2 (da=+1): q=p-1
    sm = np.zeros((128, 3, 128), dtype=BF)
    for q in range(P):
        if q - 1 >= 0:
            sm[q - 1, 0, q] = 1.0
        sm[q, 1, q] = 1.0
        if q + 1 < P:
            sm[q + 1, 2, q] = 1.0
    for c in range(NCORES):
        in_maps[c]["shm"] = sm

    for ai, a in enumerate(AXES):
        base = 8 * ai
        shifted = np.empty((8, X, Y, Z), dtype=np.float32)
        for si, (da, db) in enumerate(HOST_SLOTS):
            k = DLIST.index((da, db))
            dx, dy, dz = _axis_d(a, da, db)
            shifted[si] = _shift_full(guidance[base + k], dx, dy, dz)
        for c in range(NCORES):
            ga = np.empty((P, 8, FD), dtype=np.float32)
            for si, (da, db) in enumerate(HOST_SLOTS):
                bake = da if a in ("x", "y") else 0
                ga[:, si] = _slab(shifted[si], x0s[c] - bake)
            in_maps[c][f"g_{a}"] = ga.astype(BF)
            if a == "x":
                gu = np.empty((P, 6, FD), dtype=np.float32)
                for j, si in enumerate((0, 1, 2, 5, 6, 7)):
                    gu[:, j] = _slab(shifted[si], x0s[c])
                in_maps[c]["gux"] = gu.astype(BF)

    for c in range(NCORES):
        in_maps[c]["r0"] = _slab(blur, x0s[c]).astype(np.float32)
        for name, dx in (("rm0", -1), ("rp0", 1)):
            sl = np.zeros((P, SLOTF), dtype=BF)
            sl[:, GUARD:GUARD + FD] = _slab(blur, x0s[c] + dx).astype(BF)
            in_maps[c][name] = sl

    return in_maps


def _unswizzle(results):
    out = np.empty((1, X, Y, Z), dtype=np.float32)
    for c in range(NCORES):
        r = results[c]["rout"].reshape(P, YC, ZC)
        x0 = c * W
        for b in range(NYB):
            ys = b * YT
            ye = min(Y, ys + YT)
            out[0, x0:x0 + W, ys:ye, :] = \
                r[b * S + M: b * S + M + W, 2:2 + (ye - ys), :]
    return out


def kernel(guidance, blur):
    global _COMPILED, _LAST_RESULTS
    from concourse import bass_utils
    if _COMPILED is None:
        _COMPILED = _build_program()
    nc = _COMPILED
    in_maps = _prep_inputs(guidance, blur)
    res = bass_utils.run_bass_kernel_spmd(nc, in_maps,
                                          core_ids=list(range(NCORES)))
    _LAST_RESULTS = res
    return _unswizzle(res.results)


# revision 54
# speedup vs baseline: 1.0264x; 1.0128x over previous
"""Affinity-propagation (CSPN-3D) Trainium2 kernel, v3.

Problem: guidance [24,256,256,32] f32, blur [1,256,256,32] f32.
3 iterations of (x-plane, y-plane, z-plane) 8-neighbor gated propagation:

  out(q) = r(q) + c1(q) * [ sum_k G_k(q+d_k) * r(q+d_k) - S(q) * r(q) ]
  A(q) = sum_k |G_k(q+d_k)|, S(q) = sum_k G_k(q+d_k), c1 = 1/max(A,eps)

Sharding: 8 cores, X sharded 32 rows/core, ghost margin 5 rows each side;
step 1 consumes no margin (host supplies +-1-x-shifted blur slabs and
unbaked x gates), the remaining 5 x-crossing steps consume 1 each.

Per-core layout: partitions p = yb*42 + xl (3 y-blocks x 42 x-rows = 126);
free f = ylocal*32 + z, ylocal in [0,90) (86-wide y third + 2 overlap cols
each side, refreshed before steps 4 and 7), z in [0,32) unpadded
(z boundary handled by zero gates). FD = 2880, chunked [512 x 5, 320].

Gates are host-pre-shifted by their full 3D offset and, for the x/y axes,
additionally "baked" by -da along partitions so products are computed
against the partition-local rc copy; a PE matmul with a +-1-shift
stationary routes each da group back while accumulating all 9 slots
(8 gates + a -S center slot) into PSUM. Per step:
 - Act: rc = bf16(r) into a guarded window buffer; psum -> bf16 drain.
 - DVE: one 9-slot product instruction per chunk (windowed 4-dim AP).
 - PE: 9 matmuls (3 stationaries) accumulate slots into PSUM f32.
 - Pool: t = c1*psum_bf16 ; shadow f32 r += t (output only).
 - DVE: rc += t is the propagating bf16 state (no per-step f32->bf16
   recopy; ghost-col exchange runs in rc space).
A/S/c1 are computed on device: |g| via 4x-mode bitmask (DVE), slot sums
via the same PE routing, 1/A via DVE reciprocal.
"""

import numpy as np
import ml_dtypes

BF = ml_dtypes.bfloat16

X = Y = 256
Z = 32
NCORES = 8
W = X // NCORES          # 32 interior x rows per core
M = 5                    # ghost margin rows
S = W + 2 * M            # 42 slab rows
NYB = 3                  # y thirds
YT = 86                  # y third width
YC = YT + 4              # y cols incl 2 overlap each side
ZC = Z                   # z cols, unpadded
FD = YC * ZC             # 2880
P = NYB * S              # 126 partitions
CHUNKS = [(0, 512), (512, 512), (1024, 512), (1536, 512),
          (2048, 512), (2560, 320)]
NCH = len(CHUNKS)
GUARD = 34               # window guard (max offset ZC+1=33)
SLOTF = GUARD + FD + GUARD   # 2948
PROP_TIME = 3
EPS = 1e-30

# k -> (dH, dW) neighbor offsets, matching reference PADS
DLIST = [(1, 1), (1, 0), (1, -1), (0, 1), (0, -1), (-1, 1), (-1, 0), (-1, -1)]
# 3x3 slot enumeration (da, db), row-major; center (0,0) is the nS slot.
SLOT33 = [(-1, -1), (-1, 0), (-1, 1), (0, -1), (0, 0), (0, 1),
          (1, -1), (1, 0), (1, 1)]
HOST_SLOTS = [s for s in SLOT33 if s != (0, 0)]   # 8 real gate slots
DEV_SLOT = [0, 1, 2, 3, 5, 6, 7, 8]               # device slot of HOST_SLOTS[i]
# matmul emission order: center group (identity) first, then da=-1, da=+1
MM_ORDER = [3, 4, 5, 0, 1, 2, 6, 7, 8]
SMI = {0: 0, 1: 0, 2: 0, 3: 1, 4: 1, 5: 1, 6: 2, 7: 2, 8: 2}

AXES = ["x", "y", "z"]


def _axis_d(axis, da, db):
    if axis == "x":
        return (da, db, 0)
    if axis == "y":
        return (da, 0, db)
    return (0, da, db)


# db free-dim stride per axis (axis z: da is also free with stride ZC)
DBU = {"x": ZC, "y": 1}


def _shift_full(f, dx, dy, dz):
    """Zero-padded shift: out[x,y,z] = f[x+dx, y+dy, z+dz]."""
    o = np.zeros_like(f)
    tx0, tx1 = max(0, -dx), min(X, X - dx)
    ty0, ty1 = max(0, -dy), min(Y, Y - dy)
    tz0, tz1 = max(0, -dz), min(Z, Z - dz)
    o[tx0:tx1, ty0:ty1, tz0:tz1] = f[tx0 + dx:tx1 + dx, ty0 + dy:ty1 + dy,
                                     tz0 + dz:tz1 + dz]
    return o


def _slab(f, x0):
    """Full field [X,Y,Z] -> core slab [P, FD] (f32)."""
    pf = np.zeros((S, Y + 8, Z), dtype=np.float32)
    r0_, r1_ = x0 - M, x0 - M + S
    c0_, c1_ = max(0, r0_), min(X, r1_)
    pf[c0_ - r0_:c1_ - r0_, 2:Y + 2, :] = f[c0_:c1_]
    blocks = [pf[:, b * YT:b * YT + YC, :] for b in range(NYB)]
    return np.concatenate(blocks, axis=0).reshape(P, FD)


_COMPILED = None
_LAST_RESULTS = None


def _build_program():
    import concourse.bacc as bacc
    import concourse.mybir as mybir
    import concourse.tile as tile

    f32 = mybir.dt.float32
    bf16 = mybir.dt.bfloat16
    i16 = mybir.dt.int16
    MULT = mybir.AluOpType.mult
    AND = mybir.AluOpType.bitwise_and
    COPY = mybir.ActivationFunctionType.Copy

    nc = bacc.Bacc("TRN2", target_bir_lowering=False, debug=False,
                   num_devices=NCORES, dynamic_dma_scratch_size=2048)

    for val in (-EPS, EPS):
        ct = nc.alloc_sbuf_tensor(f"const-f32-{val}", [128, 1], f32)
        nc.gpsimd.memset(ct.ap(), val)
        nc.const_aps.aps[(f32, val)] = ct.ap()

    # ---- DRAM I/O ----
    g_in = {a: nc.dram_tensor(f"g_{a}", [P, 8, FD], bf16,
                              kind="ExternalInput").ap()
            for a in ("x", "y", "z")}
    gux_in = nc.dram_tensor("gux", [P, 6, FD], bf16,
                            kind="ExternalInput").ap()
    r0_in = nc.dram_tensor("r0", [P, FD], f32, kind="ExternalInput").ap()
    rm0_in = nc.dram_tensor("rm0", [P, SLOTF], bf16,
                            kind="ExternalInput").ap()
    rp0_in = nc.dram_tensor("rp0", [P, SLOTF], bf16,
                            kind="ExternalInput").ap()
    shm_in = nc.dram_tensor("shm", [128, 3, 128], bf16,
                            kind="ExternalInput").ap()
    rout = nc.dram_tensor("rout", [P, FD], f32, kind="ExternalOutput").ap()

    with tile.TileContext(nc) as tc:
        with tc.tile_pool(name="stat", bufs=1) as st, \
             tc.tile_pool(name="wk", bufs=1) as wk, \
             tc.tile_pool(name="fin", bufs=3) as fin, \
             tc.tile_pool(name="psum", bufs=2, space="PSUM") as pp, \
             tc.tile_pool(name="psprepA", bufs=1, space="PSUM") as pqa, \
             tc.tile_pool(name="psprepB", bufs=2, space="PSUM") as pqb:

            # ---- static tiles ----
            t_g = {a: st.tile([P, 9, FD], bf16, tag=f"g{a}", name=f"t_g{a}")
                   for a in ("x", "y")}
            t_gz = [st.tile([P, 9, 512], bf16, tag=f"gz{i}", name=f"t_gz{i}")
                    for i in range(3)]
            t_nsz = st.tile([P, FD], bf16, tag="nsz", name="t_nsz")
            t_c1 = {a: st.tile([P, FD], bf16, tag=f"c1{a}", name=f"t_c1{a}")
                    for a in AXES}
            t_r = st.tile([P, FD], f32, tag="r", name="t_r")
            t_rs = st.tile([P, 3, SLOTF], bf16, tag="rs", name="t_rs")
            t_shm = st.tile([128, 3, 128], bf16, tag="shm", name="t_shm")
            t_p = [st.tile([P, 9, 512], bf16, tag=f"p{i}", name=f"t_p{i}")
                   for i in range(2)]
            t_tt = st.tile([P, 6, 512], bf16, tag="tt6", name="t_tt")

            APc = type(t_rs[:])
            rs_ap = t_rs[:]
            rs_pd = list(rs_ap.ap[0])
            rs_base = rs_ap.offset

            def win_rc(dbu, c0, cw):
                # all 9 slots on rc: [P, 3(da: routed, stride 0),
                #                     3(db win), cw]
                off = rs_base + SLOTF + GUARD + c0 - dbu
                return APc(rs_ap.tensor, off,
                           [rs_pd, [0, 3], [dbu, 3], [1, cw]])

            def win_z(c0, cw):
                # [P, 3(dy win), 3(dz win), cw] on rc
                off = rs_base + SLOTF + GUARD + c0 - ZC - 1
                return APc(rs_ap.tensor, off,
                           [rs_pd, [ZC, 3], [1, 3], [1, cw]])

            def win_s1(u, dbu, c0, cw):
                # step 1 group u: [P, 3(db win), cw] on host buffer u
                off = rs_base + u * SLOTF + GUARD + c0 - dbu
                return APc(rs_ap.tensor, off,
                           [rs_pd, [dbu, 3], [1, cw]])

            # ---- init ----
            nc.sync.dma_start(out=t_shm[:], in_=shm_in[:])
            nc.gpsimd.memset(t_rs[:], 0.0)
            nc.sync.dma_start(out=t_r[:], in_=r0_in[:])
            nc.sync.dma_start(out=t_rs[:, 0, :], in_=rm0_in[:])
            nc.sync.dma_start(out=t_rs[:, 2, :], in_=rp0_in[:])

            def load_resident(a):
                for ci in range(NCH):
                    c0, cw = CHUNKS[ci]
                    csl = slice(c0, c0 + cw)
                    nc.scalar.dma_start(out=t_g[a][:, 0:4, csl],
                                        in_=g_in[a][:, 0:4, csl])
                    nc.scalar.dma_start(out=t_g[a][:, 5:9, csl],
                                        in_=g_in[a][:, 4:8, csl])

            preps = {}   # (a, ci) -> (psA, psS) live PSUM tiles

            def prep_s1(a, ci, gsrc, smi_of, pq):
                """Gate-normalization sums for chunk ci of axis a.
                gsrc: AP [P, 9, cw] (slots 0-3, 5-9 hold gates)."""
                c0, cw = CHUNKS[ci]
                psA = pq.tile([P, 512], f32, tag="psA", name="psA")
                psS = pq.tile([P, 512], f32, tag="psS", name="psS")
                for h, sl in ((0, slice(0, 4)), (1, slice(5, 9))):
                    tabs = wk.tile([P, 4, 512], bf16, tag="tabs", name="tabs")
                    nc.vector.tensor_scalar(
                        tabs[:, :, 0:cw].bitcast(i16),
                        gsrc[:, sl, :].bitcast(i16),
                        0x7FFF, None, AND)
                    for j in range(4):
                        s = (0, 1, 2, 3)[j] if h == 0 else (5, 6, 7, 8)[j]
                        nc.tensor.matmul(psA[:, 0:cw],
                                         t_shm[0:P, smi_of(s), 0:P],
                                         tabs[:, j, 0:cw],
                                         start=(h == 0 and j == 0),
                                         stop=(h == 1 and j == 3))
                for j, s in enumerate(DEV_SLOT):
                    nc.tensor.matmul(psS[:, 0:cw],
                                     t_shm[0:P, smi_of(s), 0:P],
                                     gsrc[:, s, :],
                                     start=(j == 0), stop=(j == 7))
                preps[(a, ci)] = (psA, psS)

            def prep_s2(a, ci, ns_dst):
                """Normalize: c1 and -S from the stage-1 sums."""
                c0, cw = CHUNKS[ci]
                csl = slice(c0, c0 + cw)
                psA, psS = preps.pop((a, ci))
                # c1 = 1/max(A, eps): Relu(A-eps)+eps is exact in f32
                tA = wk.tile([P, 512], f32, tag="tA", name="tA")
                nc.scalar.activation(tA[:, 0:cw], psA[:, 0:cw],
                                     mybir.ActivationFunctionType.Relu,
                                     bias=-EPS, scale=1.0)
                nc.scalar.activation(tA[:, 0:cw], tA[:, 0:cw],
                                     mybir.ActivationFunctionType.Identity,
                                     bias=EPS, scale=1.0)
                nc.vector.reciprocal_approx_fast(tA[:, 0:cw], tA[:, 0:cw])
                nc.scalar.activation(t_c1[a][:, csl], tA[:, 0:cw], COPY)
                # nS = -S (bf16)
                nc.scalar.activation(ns_dst, psS[:, 0:cw], COPY, scale=-1.0)

            def prep_chunk(a, ci, gsrc, ns_dst, smi_of):
                prep_s1(a, ci, gsrc, smi_of, pqa)
                prep_s2(a, ci, ns_dst)

            gchunk = [0]   # global chunk counter: t_p buffer parity

            def emit_step(step, a, pre_chunk=None, post_chunk=None,
                          zres=False):
                """One propagation step. step in 1..9."""
                zstep = a == "z"
                first = step == 1
                if step in (4, 7):
                    # y-ghost col refresh in rc space (blocks overlap by 2
                    # cols each side; y-touching steps 1,3 / 4,6 / 7,9
                    # consume one col per side between refreshes)
                    gi = GUARD
                    nc.sync.dma_start(
                        out=t_rs[S:P, 1, gi:gi + 2 * ZC],
                        in_=t_rs[0:P - S, 1,
                                 gi + YT * ZC:gi + YT * ZC + 2 * ZC])
                    nc.gpsimd.dma_start(
                        out=t_rs[0:P - S, 1, gi + FD - 2 * ZC:gi + FD],
                        in_=t_rs[S:P, 1, gi + 2 * ZC:gi + 4 * ZC])
                corder = list(range(NCH))

                def rc_update(ci):
                    c0_, cw_ = CHUNKS[ci]
                    rcc = t_rs[:, 1, GUARD + c0_:GUARD + c0_ + cw_]
                    nc.vector.tensor_tensor(
                        out=rcc, in0=t_tt[:, ci, 0:cw_], in1=rcc,
                        op=mybir.AluOpType.add)

                for pos, ci in enumerate(corder):
                    c0, cw = CHUNKS[ci]
                    csl = slice(c0, c0 + cw)
                    gchunk[0] += 1
                    if pre_chunk is not None:
                        pre_chunk(ci)
                    if zstep and not zres:
                        zbuf = pos % 3
                        if pos >= 3:   # pos 0,1,2 were prefetched
                            nc.scalar.dma_start(out=t_gz[zbuf][:, 0:4, 0:cw],
                                                in_=g_in["z"][:, 0:4, csl])
                            nc.scalar.dma_start(out=t_gz[zbuf][:, 5:9, 0:cw],
                                                in_=g_in["z"][:, 4:8, csl])
                        if step == 3:
                            prep_chunk("z", ci, t_gz[zbuf][:, :, 0:cw],
                                       t_gz[zbuf][:, 4, 0:cw], lambda s: 1)
                            nc.vector.tensor_scalar(
                                t_nsz[:, csl], t_gz[zbuf][:, 4, 0:cw],
                                1.0, None, MULT)
                        else:
                            nc.vector.tensor_scalar(
                                t_gz[zbuf][:, 4, 0:cw], t_nsz[:, csl],
                                1.0, None, MULT)
                    buf = gchunk[0] % 2
                    if zstep:
                        zin0 = (t_g["x"][:, :, csl] if zres
                                else t_gz[zbuf][:, :, 0:cw])
                        nc.vector.tensor_tensor(
                            out=t_p[buf][:, :, 0:cw]
                            .rearrange("p (u v) f -> p u v f", u=3),
                            in0=zin0.rearrange("p (u v) f -> p u v f", u=3),
                            in1=win_z(c0, cw), op=MULT)
                    elif first:
                        # stream unbaked da=+-1 groups; center from resident
                        zbuf = pos % 2
                        nc.sync.dma_start(out=t_gz[zbuf][:, 0:3, 0:cw],
                                          in_=gux_in[:, 0:3, csl])
                        nc.sync.dma_start(out=t_gz[zbuf][:, 6:9, 0:cw],
                                          in_=gux_in[:, 3:6, csl])
                        for u, src in ((0, t_gz[zbuf]), (1, t_g[a]),
                                       (2, t_gz[zbuf])):
                            if u == 1:
                                in0 = src[:, 3:6, csl]
                            else:
                                in0 = src[:, 3 * u:3 * u + 3, 0:cw]
                            nc.vector.tensor_tensor(
                                out=t_p[buf][:, 3 * u:3 * u + 3, 0:cw],
                                in0=in0,
                                in1=win_s1(u, DBU[a], c0, cw), op=MULT)
                    else:
                        nc.vector.tensor_tensor(
                            out=t_p[buf][:, :, 0:cw]
                            .rearrange("p (u v) f -> p u v f", u=3),
                            in0=t_g[a][:, :, csl]
                            .rearrange("p (u v) f -> p u v f", u=3),
                            in1=win_rc(DBU[a], c0, cw), op=MULT)
                    tps = pp.tile([P, 512], f32, tag="tps", name="tps")
                    for mi, s in enumerate(MM_ORDER):
                        smi = 1 if (first or zstep) else SMI[s]
                        nc.tensor.matmul(tps[:, 0:cw], t_shm[0:P, smi, 0:P],
                                         t_p[buf][:, s, 0:cw],
                                         start=(mi == 0), stop=(mi == 8))
                    # combine: psb = bf16(psum) [Act]; t = c1*psb [Pool];
                    # rc += t [DVE, the propagating bf16 state];
                    # r_f32 += t [Pool, off the critical path, output only]
                    psb = fin.tile([P, 512], bf16, tag="psb", name="psb")
                    nc.scalar.activation(psb[:, 0:cw], tps[:, 0:cw], COPY)
                    tt = t_tt[:, ci, :]
                    nc.gpsimd.tensor_tensor(
                        out=tt[:, 0:cw], in0=psb[:, 0:cw],
                        in1=t_c1[a][:, csl], op=MULT)
                    nc.gpsimd.tensor_tensor(
                        out=t_r[:, csl], in0=tt[:, 0:cw],
                        in1=t_r[:, csl], op=mybir.AluOpType.add)
                    if step == 9:
                        nc.sync.dma_start(out=rout[:, csl], in_=t_r[:, csl])
                    if post_chunk is not None:
                        post_chunk(ci)
                    # rc update, two positions behind: legal once both
                    # f-neighbor products are emitted (DVE runs in order, and
                    # a product's window reads the neighbor's edge cols); the
                    # extra position gives the Pool combine chain time to
                    # deliver tt without head-of-line blocking the DVE.
                    if pos >= 2:
                        rc_update(corder[pos - 2])
                rc_update(corder[-2])
                rc_update(corder[-1])

            # ---- schedule ----
            def prep_of(a):
                def f(ci):
                    c0, cw = CHUNKS[ci]
                    csl = slice(c0, c0 + cw)
                    prep_chunk(a, ci, t_g[a][:, :, csl],
                               t_g[a][:, 4, csl], lambda s: SMI[s])
                return f

            def z_prefetch(zstep):
                for pos, ci in enumerate((0, 1, 2)):
                    c0, cw = CHUNKS[ci]
                    csl = slice(c0, c0 + cw)
                    nc.scalar.dma_start(out=t_gz[pos][:, 0:4, 0:cw],
                                        in_=g_in["z"][:, 0:4, csl])
                    nc.scalar.dma_start(out=t_gz[pos][:, 5:9, 0:cw],
                                        in_=g_in["z"][:, 4:8, csl])

            def y_s1(ci):
                c0, cw = CHUNKS[ci]
                csl = slice(c0, c0 + cw)
                prep_s1("y", ci, t_g["y"][:, :, csl], lambda s: SMI[s], pqb)

            def y_s2(ci):
                c0, cw = CHUNKS[ci]
                prep_s2("y", ci, t_g["y"][:, 4, c0:c0 + cw])

            def step1_post(ci):
                # software-pipelined prep-y: loads now, sums one chunk
                # behind, normalize two behind, so the long dependency
                # chains never head-of-line-block the in-order DVE.
                c0, cw = CHUNKS[ci]
                csl = slice(c0, c0 + cw)
                nc.scalar.dma_start(out=t_g["y"][:, 0:4, csl],
                                    in_=g_in["y"][:, 0:4, csl])
                nc.scalar.dma_start(out=t_g["y"][:, 5:9, csl],
                                    in_=g_in["y"][:, 4:8, csl])
                if ci >= 1:
                    y_s1(ci - 1)
                if ci >= 2:
                    y_s2(ci - 2)

            load_resident("x")
            # prefix: rc = bf16(r0)
            for c0, cw in CHUNKS:
                nc.scalar.activation(
                    t_rs[:, 1, GUARD + c0:GUARD + c0 + cw],
                    t_r[:, c0:c0 + cw], COPY)
            # prep-x feeds step 1's combine; prep-y rides along to keep the
            # PE busy during step 1's product stalls.
            emit_step(1, "x", pre_chunk=prep_of("x"), post_chunk=step1_post)
            y_s1(NCH - 1)
            y_s2(NCH - 2)
            y_s2(NCH - 1)
            step = 1
            for it in range(PROP_TIME):
                for a in AXES:
                    if it == 0 and a == "x":
                        continue
                    step += 1
                    if a == "z" and step != 9:
                        z_prefetch(step)
                    emit_step(step, a, zres=(step == 9))
                    if step == 7:
                        # t_g["x"] is dead after step 7: make step 9's z
                        # gates fully resident there during step 8
                        for c0, cw in CHUNKS:
                            csl = slice(c0, c0 + cw)
                            nc.scalar.dma_start(out=t_g["x"][:, 0:4, csl],
                                                in_=g_in["z"][:, 0:4, csl])
                            nc.scalar.dma_start(out=t_g["x"][:, 5:9, csl],
                                                in_=g_in["z"][:, 4:8, csl])
                            nc.vector.tensor_scalar(
                                t_g["x"][:, 4, csl], t_nsz[:, csl],
                                1.0, None, MULT)

    nc.compile()
    return nc


def _prep_inputs(guidance, blur):
    """Host-side swizzle: build per-core input dicts (layout only)."""
    guidance = np.asarray(guidance, dtype=np.float32)
    blur = np.asarray(blur, dtype=np.float32)[0]  # [X,Y,Z]
    x0s = [c * W for c in range(NCORES)]

    in_maps = [dict() for _ in range(NCORES)]

    # shift matrices sm[p, g, q]: route product at partition p=q+da -> q.
    # g=0 (da=-1): q=p+1 ; g=1: q=p ; g=2 (da=+1): q=p-1
    sm = np.zeros((128, 3, 128), dtype=BF)
    for q in range(P):
        if q - 1 >= 0:
            sm[q - 1, 0, q] = 1.0
        sm[q, 1, q] = 1.0
        if q + 1 < P:
            sm[q + 1, 2, q] = 1.0
    for c in range(NCORES):
        in_maps[c]["shm"] = sm

    for ai, a in enumerate(AXES):
        base = 8 * ai
        shifted = np.empty((8, X, Y, Z), dtype=np.float32)
        for si, (da, db) in enumerate(HOST_SLOTS):
            k = DLIST.index((da, db))
            dx, dy, dz = _axis_d(a, da, db)
            shifted[si] = _shift_full(guidance[base + k], dx, dy, dz)
        for c in range(NCORES):
            ga = np.empty((P, 8, FD), dtype=np.float32)
            for si, (da, db) in enumerate(HOST_SLOTS):
                bake = da if a in ("x", "y") else 0
                ga[:, si] = _slab(shifted[si], x0s[c] - bake)
            in_maps[c][f"g_{a}"] = ga.astype(BF)
            if a == "x":
                gu = np.empty((P, 6, FD), dtype=np.float32)
                for j, si in enumerate((0, 1, 2, 5, 6, 7)):
                    gu[:, j] = _slab(shifted[si], x0s[c])
                in_maps[c]["gux"] = gu.astype(BF)

    for c in range(NCORES):
        in_maps[c]["r0"] = _slab(blur, x0s[c]).astype(np.float32)
        for name, dx in (("rm0", -1), ("rp0", 1)):
            sl = np.zeros((P, SLOTF), dtype=BF)
            sl[:, GUARD:GUARD + FD] = _slab(blur, x0s[c] + dx).astype(BF)
            in_maps[c][name] = sl

    return in_maps


def _unswizzle(results):
    out = np.empty((1, X, Y, Z), dtype=np.float32)
    for c in range(NCORES):
        r = results[c]["rout"].reshape(P, YC, ZC)
        x0 = c * W
        for b in range(NYB):
            ys = b * YT
            ye = min(Y, ys + YT)
            out[0, x0:x0 + W, ys:ye, :] = \
                r[b * S + M: b * S + M + W, 2:2 + (ye - ys), :]
    return out


def kernel(guidance, blur):
    global _COMPILED, _LAST_RESULTS
    from concourse import bass_utils
    if _COMPILED is None:
        _COMPILED = _build_program()
    nc = _COMPILED
    in_maps = _prep_inputs(guidance, blur)
    res = bass_utils.run_bass_kernel_spmd(nc, in_maps,
                                          core_ids=list(range(NCORES)))
    _LAST_RESULTS = res
    return _unswizzle(res.results)


# revision 55
# speedup vs baseline: 1.0289x; 1.0024x over previous
"""Affinity-propagation (CSPN-3D) Trainium2 kernel, v3.

Problem: guidance [24,256,256,32] f32, blur [1,256,256,32] f32.
3 iterations of (x-plane, y-plane, z-plane) 8-neighbor gated propagation:

  out(q) = r(q) + c1(q) * [ sum_k G_k(q+d_k) * r(q+d_k) - S(q) * r(q) ]
  A(q) = sum_k |G_k(q+d_k)|, S(q) = sum_k G_k(q+d_k), c1 = 1/max(A,eps)

Sharding: 8 cores, X sharded 32 rows/core, ghost margin 5 rows each side;
step 1 consumes no margin (host supplies +-1-x-shifted blur slabs and
unbaked x gates), the remaining 5 x-crossing steps consume 1 each.

Per-core layout: partitions p = yb*42 + xl (3 y-blocks x 42 x-rows = 126);
free f = ylocal*32 + z, ylocal in [0,90) (86-wide y third + 2 overlap cols
each side, refreshed before steps 4 and 7), z in [0,32) unpadded
(z boundary handled by zero gates). FD = 2880, chunked [512 x 5, 320].

Gates are host-pre-shifted by their full 3D offset and, for the x/y axes,
additionally "baked" by -da along partitions so products are computed
against the partition-local rc copy; a PE matmul with a +-1-shift
stationary routes each da group back while accumulating all 9 slots
(8 gates + a -S center slot) into PSUM. Per step:
 - Act: rc = bf16(r) into a guarded window buffer; psum -> bf16 drain.
 - DVE: one 9-slot product instruction per chunk (windowed 4-dim AP).
 - PE: 9 matmuls (3 stationaries) accumulate slots into PSUM f32.
 - Pool: t = c1*psum_bf16 ; shadow f32 r += t (output only).
 - DVE: rc += t is the propagating bf16 state (no per-step f32->bf16
   recopy; ghost-col exchange runs in rc space).
A/S/c1 are computed on device: |g| via 4x-mode bitmask (DVE), slot sums
via the same PE routing, 1/A via DVE reciprocal.
"""

import numpy as np
import ml_dtypes

BF = ml_dtypes.bfloat16

X = Y = 256
Z = 32
NCORES = 8
W = X // NCORES          # 32 interior x rows per core
M = 5                    # ghost margin rows
S = W + 2 * M            # 42 slab rows
NYB = 3                  # y thirds
YT = 86                  # y third width
YC = YT + 4              # y cols incl 2 overlap each side
ZC = Z                   # z cols, unpadded
FD = YC * ZC             # 2880
P = NYB * S              # 126 partitions
CHUNKS = [(0, 512), (512, 512), (1024, 512), (1536, 512),
          (2048, 512), (2560, 320)]
NCH = len(CHUNKS)
GUARD = 34               # window guard (max offset ZC+1=33)
SLOTF = GUARD + FD + GUARD   # 2948
PROP_TIME = 3
EPS = 1e-30

# k -> (dH, dW) neighbor offsets, matching reference PADS
DLIST = [(1, 1), (1, 0), (1, -1), (0, 1), (0, -1), (-1, 1), (-1, 0), (-1, -1)]
# 3x3 slot enumeration (da, db), row-major; center (0,0) is the nS slot.
SLOT33 = [(-1, -1), (-1, 0), (-1, 1), (0, -1), (0, 0), (0, 1),
          (1, -1), (1, 0), (1, 1)]
HOST_SLOTS = [s for s in SLOT33 if s != (0, 0)]   # 8 real gate slots
DEV_SLOT = [0, 1, 2, 3, 5, 6, 7, 8]               # device slot of HOST_SLOTS[i]
# matmul emission order: center group (identity) first, then da=-1, da=+1
MM_ORDER = [3, 4, 5, 0, 1, 2, 6, 7, 8]
SMI = {0: 0, 1: 0, 2: 0, 3: 1, 4: 1, 5: 1, 6: 2, 7: 2, 8: 2}

AXES = ["x", "y", "z"]


def _axis_d(axis, da, db):
    if axis == "x":
        return (da, db, 0)
    if axis == "y":
        return (da, 0, db)
    return (0, da, db)


# db free-dim stride per axis (axis z: da is also free with stride ZC)
DBU = {"x": ZC, "y": 1}


def _shift_full(f, dx, dy, dz):
    """Zero-padded shift: out[x,y,z] = f[x+dx, y+dy, z+dz]."""
    o = np.zeros_like(f)
    tx0, tx1 = max(0, -dx), min(X, X - dx)
    ty0, ty1 = max(0, -dy), min(Y, Y - dy)
    tz0, tz1 = max(0, -dz), min(Z, Z - dz)
    o[tx0:tx1, ty0:ty1, tz0:tz1] = f[tx0 + dx:tx1 + dx, ty0 + dy:ty1 + dy,
                                     tz0 + dz:tz1 + dz]
    return o


def _slab(f, x0):
    """Full field [X,Y,Z] -> core slab [P, FD] (f32)."""
    pf = np.zeros((S, Y + 8, Z), dtype=np.float32)
    r0_, r1_ = x0 - M, x0 - M + S
    c0_, c1_ = max(0, r0_), min(X, r1_)
    pf[c0_ - r0_:c1_ - r0_, 2:Y + 2, :] = f[c0_:c1_]
    blocks = [pf[:, b * YT:b * YT + YC, :] for b in range(NYB)]
    return np.concatenate(blocks, axis=0).reshape(P, FD)


_COMPILED = None
_LAST_RESULTS = None


def _build_program():
    import concourse.bacc as bacc
    import concourse.mybir as mybir
    import concourse.tile as tile

    f32 = mybir.dt.float32
    bf16 = mybir.dt.bfloat16
    i16 = mybir.dt.int16
    MULT = mybir.AluOpType.mult
    AND = mybir.AluOpType.bitwise_and
    COPY = mybir.ActivationFunctionType.Copy

    nc = bacc.Bacc("TRN2", target_bir_lowering=False, debug=False,
                   num_devices=NCORES, dynamic_dma_scratch_size=2048)

    for val in (-EPS, EPS):
        ct = nc.alloc_sbuf_tensor(f"const-f32-{val}", [128, 1], f32)
        nc.gpsimd.memset(ct.ap(), val)
        nc.const_aps.aps[(f32, val)] = ct.ap()

    # ---- DRAM I/O ----
    g_in = {a: nc.dram_tensor(f"g_{a}", [P, 8, FD], bf16,
                              kind="ExternalInput").ap()
            for a in ("x", "y", "z")}
    gux_in = nc.dram_tensor("gux", [P, 6, FD], bf16,
                            kind="ExternalInput").ap()
    r0_in = nc.dram_tensor("r0", [P, FD], f32, kind="ExternalInput").ap()
    rm0_in = nc.dram_tensor("rm0", [P, SLOTF], bf16,
                            kind="ExternalInput").ap()
    rp0_in = nc.dram_tensor("rp0", [P, SLOTF], bf16,
                            kind="ExternalInput").ap()
    shm_in = nc.dram_tensor("shm", [128, 3, 128], bf16,
                            kind="ExternalInput").ap()
    rout = nc.dram_tensor("rout", [P, FD], f32, kind="ExternalOutput").ap()

    with tile.TileContext(nc) as tc:
        with tc.tile_pool(name="stat", bufs=1) as st, \
             tc.tile_pool(name="wk", bufs=1) as wk, \
             tc.tile_pool(name="fin", bufs=3) as fin, \
             tc.tile_pool(name="psum", bufs=2, space="PSUM") as pp, \
             tc.tile_pool(name="psprepA", bufs=1, space="PSUM") as pqa, \
             tc.tile_pool(name="psprepB", bufs=2, space="PSUM") as pqb:

            # ---- static tiles ----
            t_g = {a: st.tile([P, 9, FD], bf16, tag=f"g{a}", name=f"t_g{a}")
                   for a in ("x", "y")}
            t_gz = [st.tile([P, 9, 512], bf16, tag=f"gz{i}", name=f"t_gz{i}")
                    for i in range(3)]
            t_nsz = st.tile([P, FD], bf16, tag="nsz", name="t_nsz")
            t_c1 = {a: st.tile([P, FD], bf16, tag=f"c1{a}", name=f"t_c1{a}")
                    for a in AXES}
            t_r = st.tile([P, FD], f32, tag="r", name="t_r")
            t_rs = st.tile([P, 3, SLOTF], bf16, tag="rs", name="t_rs")
            t_shm = st.tile([128, 3, 128], bf16, tag="shm", name="t_shm")
            t_p = [st.tile([P, 9, 512], bf16, tag=f"p{i}", name=f"t_p{i}")
                   for i in range(2)]
            t_tt = st.tile([P, 6, 512], bf16, tag="tt6", name="t_tt")

            APc = type(t_rs[:])
            rs_ap = t_rs[:]
            rs_pd = list(rs_ap.ap[0])
            rs_base = rs_ap.offset

            def win_rc(dbu, c0, cw):
                # all 9 slots on rc: [P, 3(da: routed, stride 0),
                #                     3(db win), cw]
                off = rs_base + SLOTF + GUARD + c0 - dbu
                return APc(rs_ap.tensor, off,
                           [rs_pd, [0, 3], [dbu, 3], [1, cw]])

            def win_z(c0, cw):
                # [P, 3(dy win), 3(dz win), cw] on rc
                off = rs_base + SLOTF + GUARD + c0 - ZC - 1
                return APc(rs_ap.tensor, off,
                           [rs_pd, [ZC, 3], [1, 3], [1, cw]])

            def win_s1(u, dbu, c0, cw):
                # step 1 group u: [P, 3(db win), cw] on host buffer u
                off = rs_base + u * SLOTF + GUARD + c0 - dbu
                return APc(rs_ap.tensor, off,
                           [rs_pd, [dbu, 3], [1, cw]])

            # ---- init ----
            nc.sync.dma_start(out=t_shm[:], in_=shm_in[:])
            nc.gpsimd.memset(t_rs[:], 0.0)
            nc.sync.dma_start(out=t_r[:], in_=r0_in[:])
            nc.sync.dma_start(out=t_rs[:, 0, :], in_=rm0_in[:])
            nc.sync.dma_start(out=t_rs[:, 2, :], in_=rp0_in[:])

            def load_resident(a):
                for ci in range(NCH):
                    c0, cw = CHUNKS[ci]
                    csl = slice(c0, c0 + cw)
                    nc.scalar.dma_start(out=t_g[a][:, 0:4, csl],
                                        in_=g_in[a][:, 0:4, csl])
                    nc.scalar.dma_start(out=t_g[a][:, 5:9, csl],
                                        in_=g_in[a][:, 4:8, csl])

            preps = {}   # (a, ci) -> (psA, psS) live PSUM tiles

            def prep_s1(a, ci, gsrc, smi_of, pq):
                """Gate-normalization sums for chunk ci of axis a.
                gsrc: AP [P, 9, cw] (slots 0-3, 5-9 hold gates)."""
                c0, cw = CHUNKS[ci]
                psA = pq.tile([P, 512], f32, tag="psA", name="psA")
                psS = pq.tile([P, 512], f32, tag="psS", name="psS")
                for h, sl in ((0, slice(0, 4)), (1, slice(5, 9))):
                    tabs = wk.tile([P, 4, 512], bf16, tag="tabs", name="tabs")
                    nc.vector.tensor_scalar(
                        tabs[:, :, 0:cw].bitcast(i16),
                        gsrc[:, sl, :].bitcast(i16),
                        0x7FFF, None, AND)
                    for j in range(4):
                        s = (0, 1, 2, 3)[j] if h == 0 else (5, 6, 7, 8)[j]
                        nc.tensor.matmul(psA[:, 0:cw],
                                         t_shm[0:P, smi_of(s), 0:P],
                                         tabs[:, j, 0:cw],
                                         start=(h == 0 and j == 0),
                                         stop=(h == 1 and j == 3))
                for j, s in enumerate(DEV_SLOT):
                    nc.tensor.matmul(psS[:, 0:cw],
                                     t_shm[0:P, smi_of(s), 0:P],
                                     gsrc[:, s, :],
                                     start=(j == 0), stop=(j == 7))
                preps[(a, ci)] = (psA, psS)

            def prep_s2(a, ci, ns_dst):
                """Normalize: c1 and -S from the stage-1 sums."""
                c0, cw = CHUNKS[ci]
                csl = slice(c0, c0 + cw)
                psA, psS = preps.pop((a, ci))
                # c1 = 1/max(A, eps): Relu(A-eps)+eps is exact in f32
                tA = wk.tile([P, 512], f32, tag="tA", name="tA")
                nc.scalar.activation(tA[:, 0:cw], psA[:, 0:cw],
                                     mybir.ActivationFunctionType.Relu,
                                     bias=-EPS, scale=1.0)
                nc.scalar.activation(tA[:, 0:cw], tA[:, 0:cw],
                                     mybir.ActivationFunctionType.Identity,
                                     bias=EPS, scale=1.0)
                nc.vector.reciprocal_approx_fast(tA[:, 0:cw], tA[:, 0:cw])
                nc.scalar.activation(t_c1[a][:, csl], tA[:, 0:cw], COPY)
                # nS = -S (bf16)
                nc.scalar.activation(ns_dst, psS[:, 0:cw], COPY, scale=-1.0)

            def prep_chunk(a, ci, gsrc, ns_dst, smi_of):
                prep_s1(a, ci, gsrc, smi_of, pqa)
                prep_s2(a, ci, ns_dst)

            gchunk = [0]   # global chunk counter: t_p buffer parity

            def emit_step(step, a, pre_chunk=None, post_chunk=None,
                          zres=False):
                """One propagation step. step in 1..9."""
                zstep = a == "z"
                first = step == 1
                if step in (4, 7):
                    # y-ghost col refresh in rc space (blocks overlap by 2
                    # cols each side; y-touching steps 1,3 / 4,6 / 7,9
                    # consume one col per side between refreshes)
                    gi = GUARD
                    nc.sync.dma_start(
                        out=t_rs[S:P, 1, gi:gi + 2 * ZC],
                        in_=t_rs[0:P - S, 1,
                                 gi + YT * ZC:gi + YT * ZC + 2 * ZC])
                    nc.gpsimd.dma_start(
                        out=t_rs[0:P - S, 1, gi + FD - 2 * ZC:gi + FD],
                        in_=t_rs[S:P, 1, gi + 2 * ZC:gi + 4 * ZC])
                corder = list(range(NCH))

                def rc_update(ci):
                    c0_, cw_ = CHUNKS[ci]
                    rcc = t_rs[:, 1, GUARD + c0_:GUARD + c0_ + cw_]
                    nc.vector.tensor_tensor(
                        out=rcc, in0=t_tt[:, ci, 0:cw_], in1=rcc,
                        op=mybir.AluOpType.add)

                for pos, ci in enumerate(corder):
                    c0, cw = CHUNKS[ci]
                    csl = slice(c0, c0 + cw)
                    gchunk[0] += 1
                    if pre_chunk is not None:
                        pre_chunk(ci)
                    if zstep and not zres:
                        zbuf = pos % 3
                        if pos >= 3:   # pos 0,1,2 were prefetched
                            nc.scalar.dma_start(out=t_gz[zbuf][:, 0:4, 0:cw],
                                                in_=g_in["z"][:, 0:4, csl])
                            nc.scalar.dma_start(out=t_gz[zbuf][:, 5:9, 0:cw],
                                                in_=g_in["z"][:, 4:8, csl])
                        if step == 3:
                            prep_chunk("z", ci, t_gz[zbuf][:, :, 0:cw],
                                       t_gz[zbuf][:, 4, 0:cw], lambda s: 1)
                            nc.vector.tensor_scalar(
                                t_nsz[:, csl], t_gz[zbuf][:, 4, 0:cw],
                                1.0, None, MULT)
                        else:
                            nc.vector.tensor_scalar(
                                t_gz[zbuf][:, 4, 0:cw], t_nsz[:, csl],
                                1.0, None, MULT)
                    buf = gchunk[0] % 2
                    if zstep:
                        zin0 = (t_g["x"][:, :, csl] if zres
                                else t_gz[zbuf][:, :, 0:cw])
                        nc.vector.tensor_tensor(
                            out=t_p[buf][:, :, 0:cw]
                            .rearrange("p (u v) f -> p u v f", u=3),
                            in0=zin0.rearrange("p (u v) f -> p u v f", u=3),
                            in1=win_z(c0, cw), op=MULT)
                    elif first:
                        # stream unbaked da=+-1 groups; center from resident
                        zbuf = pos % 2
                        nc.sync.dma_start(out=t_gz[zbuf][:, 0:3, 0:cw],
                                          in_=gux_in[:, 0:3, csl])
                        nc.sync.dma_start(out=t_gz[zbuf][:, 6:9, 0:cw],
                                          in_=gux_in[:, 3:6, csl])
                        for u, src in ((0, t_gz[zbuf]), (1, t_g[a]),
                                       (2, t_gz[zbuf])):
                            if u == 1:
                                in0 = src[:, 3:6, csl]
                            else:
                                in0 = src[:, 3 * u:3 * u + 3, 0:cw]
                            nc.vector.tensor_tensor(
                                out=t_p[buf][:, 3 * u:3 * u + 3, 0:cw],
                                in0=in0,
                                in1=win_s1(u, DBU[a], c0, cw), op=MULT)
                    else:
                        nc.vector.tensor_tensor(
                            out=t_p[buf][:, :, 0:cw]
                            .rearrange("p (u v) f -> p u v f", u=3),
                            in0=t_g[a][:, :, csl]
                            .rearrange("p (u v) f -> p u v f", u=3),
                            in1=win_rc(DBU[a], c0, cw), op=MULT)
                    tps = pp.tile([P, 512], f32, tag="tps", name="tps")
                    for mi, s in enumerate(MM_ORDER):
                        smi = 1 if (first or zstep) else SMI[s]
                        nc.tensor.matmul(tps[:, 0:cw], t_shm[0:P, smi, 0:P],
                                         t_p[buf][:, s, 0:cw],
                                         start=(mi == 0), stop=(mi == 8))
                    # combine: psb = bf16(psum) [Act]; t = c1*psb [Pool];
                    # rc += t [DVE, the propagating bf16 state];
                    # r_f32 += t [Pool, off the critical path, output only]
                    psb = fin.tile([P, 512], bf16, tag="psb", name="psb")
                    nc.scalar.activation(psb[:, 0:cw], tps[:, 0:cw], COPY)
                    tt = t_tt[:, ci, :]
                    nc.gpsimd.tensor_tensor(
                        out=tt[:, 0:cw], in0=psb[:, 0:cw],
                        in1=t_c1[a][:, csl], op=MULT)
                    nc.gpsimd.tensor_tensor(
                        out=t_r[:, csl], in0=tt[:, 0:cw],
                        in1=t_r[:, csl], op=mybir.AluOpType.add)
                    if step == 9:
                        nc.sync.dma_start(out=rout[:, csl], in_=t_r[:, csl])
                    if post_chunk is not None:
                        post_chunk(ci)
                    # rc update, two positions behind: legal once both
                    # f-neighbor products are emitted (DVE runs in order, and
                    # a product's window reads the neighbor's edge cols); the
                    # extra position gives the Pool combine chain time to
                    # deliver tt without head-of-line blocking the DVE.
                    if pos >= 2 and step != 9:
                        # rc is dead after the last step: skip its updates
                        rc_update(corder[pos - 2])
                if step != 9:
                    rc_update(corder[-2])
                    rc_update(corder[-1])

            # ---- schedule ----
            def prep_of(a):
                def f(ci):
                    c0, cw = CHUNKS[ci]
                    csl = slice(c0, c0 + cw)
                    prep_chunk(a, ci, t_g[a][:, :, csl],
                               t_g[a][:, 4, csl], lambda s: SMI[s])
                return f

            def z_prefetch(zstep):
                for pos, ci in enumerate((0, 1, 2)):
                    c0, cw = CHUNKS[ci]
                    csl = slice(c0, c0 + cw)
                    nc.scalar.dma_start(out=t_gz[pos][:, 0:4, 0:cw],
                                        in_=g_in["z"][:, 0:4, csl])
                    nc.scalar.dma_start(out=t_gz[pos][:, 5:9, 0:cw],
                                        in_=g_in["z"][:, 4:8, csl])

            def y_s1(ci):
                c0, cw = CHUNKS[ci]
                csl = slice(c0, c0 + cw)
                prep_s1("y", ci, t_g["y"][:, :, csl], lambda s: SMI[s], pqb)

            def y_s2(ci):
                c0, cw = CHUNKS[ci]
                prep_s2("y", ci, t_g["y"][:, 4, c0:c0 + cw])

            def step1_post(ci):
                # software-pipelined prep-y: loads now, sums one chunk
                # behind, normalize two behind, so the long dependency
                # chains never head-of-line-block the in-order DVE.
                c0, cw = CHUNKS[ci]
                csl = slice(c0, c0 + cw)
                nc.scalar.dma_start(out=t_g["y"][:, 0:4, csl],
                                    in_=g_in["y"][:, 0:4, csl])
                nc.scalar.dma_start(out=t_g["y"][:, 5:9, csl],
                                    in_=g_in["y"][:, 4:8, csl])
                if ci >= 1:
                    y_s1(ci - 1)
                if ci >= 2:
                    y_s2(ci - 2)

            load_resident("x")
            # prefix: rc = bf16(r0)
            for c0, cw in CHUNKS:
                nc.scalar.activation(
                    t_rs[:, 1, GUARD + c0:GUARD + c0 + cw],
                    t_r[:, c0:c0 + cw], COPY)
            # prep-x feeds step 1's combine; prep-y rides along to keep the
            # PE busy during step 1's product stalls.
            emit_step(1, "x", pre_chunk=prep_of("x"), post_chunk=step1_post)
            y_s1(NCH - 1)
            y_s2(NCH - 2)
            y_s2(NCH - 1)
            step = 1
            for it in range(PROP_TIME):
                for a in AXES:
                    if it == 0 and a == "x":
                        continue
                    step += 1
                    if a == "z" and step != 9:
                        z_prefetch(step)
                    emit_step(step, a, zres=(step == 9))
                    if step == 7:
                        # t_g["x"] is dead after step 7: make step 9's z
                        # gates fully resident there during step 8
                        for c0, cw in CHUNKS:
                            csl = slice(c0, c0 + cw)
                            nc.scalar.dma_start(out=t_g["x"][:, 0:4, csl],
                                                in_=g_in["z"][:, 0:4, csl])
                            nc.scalar.dma_start(out=t_g["x"][:, 5:9, csl],
                                                in_=g_in["z"][:, 4:8, csl])
                            nc.vector.tensor_scalar(
                                t_g["x"][:, 4, csl], t_nsz[:, csl],
                                1.0, None, MULT)

    nc.compile()
    return nc


def _prep_inputs(guidance, blur):
    """Host-side swizzle: build per-core input dicts (layout only)."""
    guidance = np.asarray(guidance, dtype=np.float32)
    blur = np.asarray(blur, dtype=np.float32)[0]  # [X,Y,Z]
    x0s = [c * W for c in range(NCORES)]

    in_maps = [dict() for _ in range(NCORES)]

    # shift matrices sm[p, g, q]: route product at partition p=q+da -> q.
    # g=0 (da=-1): q=p+1 ; g=1: q=p ; g=2 (da=+1): q=p-1
    sm = np.zeros((128, 3, 128), dtype=BF)
    for q in range(P):
        if q - 1 >= 0:
            sm[q - 1, 0, q] = 1.0
        sm[q, 1, q] = 1.0
        if q + 1 < P:
            sm[q + 1, 2, q] = 1.0
    for c in range(NCORES):
        in_maps[c]["shm"] = sm

    for ai, a in enumerate(AXES):
        base = 8 * ai
        shifted = np.empty((8, X, Y, Z), dtype=np.float32)
        for si, (da, db) in enumerate(HOST_SLOTS):
            k = DLIST.index((da, db))
            dx, dy, dz = _axis_d(a, da, db)
            shifted[si] = _shift_full(guidance[base + k], dx, dy, dz)
        for c in range(NCORES):
            ga = np.empty((P, 8, FD), dtype=np.float32)
            for si, (da, db) in enumerate(HOST_SLOTS):
                bake = da if a in ("x", "y") else 0
                ga[:, si] = _slab(shifted[si], x0s[c] - bake)
            in_maps[c][f"g_{a}"] = ga.astype(BF)
            if a == "x":
                gu = np.empty((P, 6, FD), dtype=np.float32)
                for j, si in enumerate((0, 1, 2, 5, 6, 7)):
                    gu[:, j] = _slab(shifted[si], x0s[c])
                in_maps[c]["gux"] = gu.astype(BF)

    for c in range(NCORES):
        in_maps[c]["r0"] = _slab(blur, x0s[c]).astype(np.float32)
        for name, dx in (("rm0", -1), ("rp0", 1)):
            sl = np.zeros((P, SLOTF), dtype=BF)
            sl[:, GUARD:GUARD + FD] = _slab(blur, x0s[c] + dx).astype(BF)
            in_maps[c][name] = sl

    return in_maps


def _unswizzle(results):
    out = np.empty((1, X, Y, Z), dtype=np.float32)
    for c in range(NCORES):
        r = results[c]["rout"].reshape(P, YC, ZC)
        x0 = c * W
        for b in range(NYB):
            ys = b * YT
            ye = min(Y, ys + YT)
            out[0, x0:x0 + W, ys:ye, :] = \
                r[b * S + M: b * S + M + W, 2:2 + (ye - ys), :]
    return out


def kernel(guidance, blur):
    global _COMPILED, _LAST_RESULTS
    from concourse import bass_utils
    if _COMPILED is None:
        _COMPILED = _build_program()
    nc = _COMPILED
    in_maps = _prep_inputs(guidance, blur)
    res = bass_utils.run_bass_kernel_spmd(nc, in_maps,
                                          core_ids=list(range(NCORES)))
    _LAST_RESULTS = res
    return _unswizzle(res.results)
